# revision 13
# baseline (speedup 1.0000x reference)
"""Causal multi-head attention (B=4, S=2048, E=768, N=12 heads, H=64) on 8
Trainium2 NeuronCores.

Sharding: core c handles batch c//2 and heads (c%2)*6 .. +6 (tensor parallel
over heads within a batch pair). No collectives: each core emits a partial
out^T = (sum over its 6 heads of z @ W_O) + b_O/2, and the host sums the two
partials per batch and transposes back.

Layout: all device math runs in a transposed layout (seq on the free axis):
  xT [E, S] per batch (host-packed into per-tile contiguous layout)
  Q^T/K^T per head-pair  [128 (2x64h), S] in per-512-column tiles
  V natural [S, 128*6] (col 0 per head is all-ones -> PV matmul row 0
                        accumulates the softmax denominator for free; value
                        columns sit at 64-127 so the z rows of the PV PSUM
                        start at partition 64 — engine partition accesses
                        must be 32-aligned; the ones/dead columns are memset
                        once, V projection writes only cols 64-127 through a
                        strided AP)
  S^T [k, q] scores, both heads of a pair computed concurrently in the PE
  array via tile_position row groups; on diagonal blocks the moving range of
  QK/PV and the exp width are restricted to the causally-live columns and
  only the 128-wide triangle chunk is masked (one [128,2,128] multiply on
  GPSIMD against a host tri-mask); P = exp(scale*S^T); z^T [64, q] is
  normalized straight out of PSUM: 1/den via DVE fast reciprocal on the
  PSUM den row, gpsimd partition_broadcast, one DVE multiply PSUM->SBUF;
  out^T [E, S] accumulated over head pairs (K=128 contraction), written bf16
  and summed/bias'd on the host.

Engine budget: ACT runs ONLY the exp stream (plus the qb=0 K/Q copies that
land in its idle startup window); all other K/Q copy-outs are DVE
tensor_scalar_add with the fused bias, V and out-projection copy-outs are
DVE, diagonal masks and the reciprocal broadcast run on GPSIMD. b_V is
folded into b_O on the host (out += P@(v+bV)/den @ Wo == out + Wo@bV since
sum(P)/den==1) and b_O is applied on the host after the partial-sum gather.

DMAs: every DRAM tensor is host-prepacked to the exact SBUF tile layout so
each transfer is one fully-contiguous descriptor run (3KB+ per partition
line). Startup loads are spread over the gpsimd/scalar/vector/sync queues so
the first K-projection starts ~1.5us in; the scalar queue is idle from the
first exp onward.

Scheduling: window qb drains the K/Q projection chains for qb+1 between the
attention matmuls (V chains run at window end); the output projections are
deferred to the LAST window (qb=3), where the exp stream runs ~20us longer
than the PE work, so they fill PE idle instead of stalling the early
ACT-starved windows.
"""

import sys

sys.path.insert(0, "/opt/trn_rl_repo")

import numpy as np

B, S, E = 4, 2048, 768
N_HEADS, H = 12, 64
HPC = 6           # heads per core
PAIRS = 3         # head pairs per core
EC = E // 128     # 6 e-chunks
QB = 512          # query block (free dim of most matmuls)
NQB = S // QB     # 4
KB = 128          # key sub-block (partition dim of S^T)
SC = S // 128     # 16 s-chunks for V
VW = 128          # V width per head: col 0 = ones (denominator), 64-127 = values
VH = 64           # value columns per head
VO = 64           # value column offset within the per-head block
SCALE = 1.0 / np.sqrt(np.float32(H))

COMPUTE_DT = "bfloat16"

_g = {"nc": None}


def _np_dt():
    if COMPUTE_DT == "bfloat16":
        import ml_dtypes

        return ml_dtypes.bfloat16
    return np.float32


def _build(num_devices=8):
    from concourse import bacc, tile, mybir, library_config

    F32 = mybir.dt.float32
    DT = getattr(mybir.dt, COMPUTE_DT)

    nc = bacc.Bacc("TRN2", target_bir_lowering=False, debug=False,
                   num_devices=num_devices)

    # host-prepacked, per-tile contiguous layouts
    d_x = nc.dram_tensor("xp", [128, NQB * 2 * 3 * QB], DT,
                         kind="ExternalInput").ap()
    d_wq = nc.dram_tensor("wq", [128, PAIRS * E], DT, kind="ExternalInput").ap()
    d_wk = nc.dram_tensor("wk", [128, PAIRS * E], DT, kind="ExternalInput").ap()
    d_wv = nc.dram_tensor("wv", [128, VH * HPC * EC], DT, kind="ExternalInput").ap()
    d_wo = nc.dram_tensor("wo", [128, PAIRS * E], DT, kind="ExternalInput").ap()
    # bundle cols: 0-2 bQ per pair, 3-5 bK per pair, 6-11 effective bO per e
    d_bundle = nc.dram_tensor("bundle", [128, 12], F32, kind="ExternalInput").ap()
    d_mask = nc.dram_tensor("mask", [KB, 2 * KB], DT, kind="ExternalInput").ap()
    d_out = nc.dram_tensor("outT", [E, S], DT, kind="ExternalOutput").ap()

    Exp = mybir.ActivationFunctionType.Exp
    Iden = mybir.ActivationFunctionType.Identity

    with tile.TileContext(nc) as tc:
        with tc.tile_pool(name="persist", bufs=1) as pp, \
             tc.tile_pool(name="work", bufs=4) as wp, \
             tc.tile_pool(name="zsb", bufs=3) as zp, \
             tc.tile_pool(name="outsb", bufs=4) as op, \
             tc.tile_pool(name="psA", bufs=1, space="PSUM") as psA:

            nc.gpsimd.load_library(library_config.proxy)

            # ---- input DMAs --------------------------------------------------
            # First-needed pieces land first on the three DMA-capable queues:
            #   gpsimd: wk p0 | wv e0-2 | wk p1/p2 | wv e3-5
            #   scalar: bundle | wq p0 | wq p1/p2  (idle before the first exp)
            #   sync:   x q0 | mask | x q1..q3 | wo (wo only needed at qb=3)
            wk_all = pp.tile([128, PAIRS * E], DT, tag="wk", name="wk_all")
            wq_all = pp.tile([128, PAIRS * E], DT, tag="wq", name="wq_all")
            wv_all = pp.tile([128, VH * HPC * EC], DT, tag="wv", name="wv_all")
            wo_all = pp.tile([128, PAIRS * E], DT, tag="wo", name="wo_all")
            bundle = pp.tile([128, 12], F32, tag="bundle", name="bundle")
            masksb = pp.tile([KB, 2 * KB], DT, tag="mask", name="masksb")

            nc.gpsimd.dma_start(wk_all[:, 0:E], d_wk[:, 0:E])
            nc.scalar.dma_start(bundle[:], d_bundle[:, :])
            nc.scalar.dma_start(wq_all[:, 0:E], d_wq[:, 0:E])
            nc.gpsimd.dma_start(wv_all[:, 0:3 * VH * HPC],
                                d_wv[:, 0:3 * VH * HPC])
            nc.scalar.dma_start(wq_all[:, E:3 * E], d_wq[:, E:3 * E])
            nc.gpsimd.dma_start(wk_all[:, E:3 * E], d_wk[:, E:3 * E])
            nc.gpsimd.dma_start(wv_all[:, 3 * VH * HPC:], d_wv[:, 3 * VH * HPC:])

            wk = [wk_all[:, p * E:(p + 1) * E] for p in range(PAIRS)]
            wq = [wq_all[:, p * E:(p + 1) * E] for p in range(PAIRS)]
            wo = [wo_all[:, p * E:(p + 1) * E] for p in range(PAIRS)]
            wv = [wv_all[:, e * VH * HPC:(e + 1) * VH * HPC] for e in range(EC)]
            mask3 = masksb[:].rearrange("p (h s) -> p h s", h=2)

            # x: one contiguous DMA per (quarter, half-of-e-chunks) tile.
            xq = [[None, None] for _ in range(4)]

            def x_dma(quarter, half):
                t = pp.tile([128, 3 * QB], DT, tag=f"xq{quarter}_{half}",
                            name=f"xq{quarter}_{half}")
                base = (quarter * 2 + half) * 3 * QB
                nc.sync.dma_start(t[:], d_x[:, base:base + 3 * QB])
                xq[quarter][half] = t

            x_dma(0, 0)
            x_dma(0, 1)
            nc.sync.dma_start(masksb[:], d_mask[:, :])
            for quarter in range(1, 4):
                x_dma(quarter, 0)
                x_dma(quarter, 1)
            nc.sync.dma_start(wo_all[:], d_wo[:, :])

            # HAM warm-up: dummy matmuls during the input-DMA wait so the
            # real stream starts at 2.4GHz instead of the cold 1.2.
            warm = pp.tile([128, QB], DT, tag="warm", name="warm")
            nc.vector.memset(warm[:], 0.0)
            for i in range(6):
                wps = psA.tile([128, QB], F32, tag="misc", bufs=2,
                               name=f"warm{i}")
                nc.tensor.matmul(wps[:], warm[:, 0:128], warm[:],
                                 start=True, stop=True)

            def xchunk(e, sb, lo=0, w=QB):
                # [128, w] slice of e-chunk e, query block sb
                base = (e % 3) * QB + lo
                return xq[sb][e // 3][:, base:base + w]

            kt = [[pp.tile([128, QB], DT, tag=f"kt{p}_{sb}", name=f"kt{p}_{sb}")
                   for sb in range(NQB)] for p in range(PAIRS)]
            qt = [[pp.tile([128, QB], DT, tag=f"qt{p}_{sb}", name=f"qt{p}_{sb}")
                   for sb in range(NQB)] for p in range(PAIRS)]
            vt = [pp.tile([128, VW * HPC], DT, tag=f"vt{s}", name=f"vt{s}")
                  for s in range(SC)]
            # ones columns for the denominator trick: memset only col 0 of
            # each per-head block (cols 1-63 feed PSUM rows that are never
            # read); projections only ever write the 64 value columns.
            for s in range(SC):
                nc.vector.memset(
                    vt[s][:].rearrange("p (h w) -> p h w", w=VW)[:, :, 0:1],
                    1.0)

            def _mk_chain():
                def chain(name, width, lhs_of_e, rhs_of_e, copy_out):
                    st = {}
                    def mk(e):
                        def step():
                            if e == 0:
                                st["ps"] = psA.tile(
                                    [128, width], F32, tag="misc", bufs=2,
                                    name=name)
                            nc.tensor.matmul(st["ps"][:],
                                             lhs_of_e(e), rhs_of_e(e),
                                             start=(e == 0), stop=(e == EC - 1))
                        return step
                    for e in range(EC):
                        yield mk(e)
                    yield lambda: copy_out(st["ps"])
                return chain

            def kq_pair_ops(sb, p, chain=None):
                chain = chain or _mk_chain()
                if sb == 0:
                    # startup window: ACT is otherwise idle here
                    kcopy = lambda ps, p=p, sb=sb: nc.scalar.activation(
                        kt[p][sb][:], ps[:], Iden, bias=bundle[:, 3 + p:4 + p])
                    qcopy = lambda ps, p=p, sb=sb: nc.scalar.activation(
                        qt[p][sb][:], ps[:], Iden, bias=bundle[:, p:p + 1])
                else:
                    kcopy = lambda ps, p=p, sb=sb: nc.vector.tensor_scalar_add(
                        kt[p][sb][:], ps[:], bundle[:, 3 + p:4 + p])
                    qcopy = lambda ps, p=p, sb=sb: nc.vector.tensor_scalar_add(
                        qt[p][sb][:], ps[:], bundle[:, p:p + 1])
                yield from chain(
                    f"kps{p}_{sb}", QB,
                    lambda e, p=p: wk[p][:, e * 128:(e + 1) * 128],
                    lambda e, sb=sb: xchunk(e, sb), kcopy)
                yield from chain(
                    f"qps{p}_{sb}", QB,
                    lambda e, p=p: wq[p][:, e * 128:(e + 1) * 128],
                    lambda e, sb=sb: xchunk(e, sb), qcopy)

            def kq_ops(sb, chain=None):
                for p in range(PAIRS):
                    yield from kq_pair_ops(sb, p, chain)

            def v_ops(sb, chain=None):
                chain = chain or _mk_chain()
                for s in range(4 * sb, 4 * sb + 4):
                    def vcopy(ps, s=s, sb=sb):
                        dst = vt[s][:].rearrange(
                            "p (h w) -> p h w", w=VW)[:, :, VO:VO + VH]
                        srcv = ps[:].rearrange("p (h w) -> p h w", w=VH)
                        if sb == 0:
                            nc.scalar.activation(dst, srcv, Iden)
                        else:
                            nc.vector.tensor_copy(dst, srcv)
                    yield from chain(
                        f"vps{s}", VH * HPC,
                        lambda e, sb=sb, s=s: xchunk(e, sb, (s % 4) * 128, 128),
                        lambda e: wv[e], vcopy)

            def make_normalize(qb, zpair):
                def release(head, zab):
                    # Emitted right at the pair end so the zab PSUM banks
                    # free before the next pair's first PV (tag z rotates 2
                    # buffers per pair): one f32 copy of the z rows plus the
                    # fast reciprocal of the PSUM den row (partition 0, as
                    # reciprocal_approx_fast requires). The broadcast runs on
                    # gpsimd immediately (nothing queued behind it); the
                    # normalize multiply is deferred so the DVE queue never
                    # head-of-line blocks on the broadcast.
                    zsb = wp.tile([VH, QB], F32, tag="zc", bufs=6,
                                  name=f"zsb{qb}_{head}")
                    nc.vector.tensor_copy(zsb[:], zab[VO:VO + VH, :])
                    recipf = wp.tile([1, QB], F32, tag="recipf", bufs=6,
                                     name=f"recipf{qb}_{head}")
                    nc.vector.reciprocal_approx_fast(recipf[:], zab[0:1, :])
                    bcast = wp.tile([64, QB], F32, tag="bcast", bufs=6,
                                    name=f"bcast{qb}_{head}")
                    nc.gpsimd.partition_broadcast(bcast[:], recipf[:])
                    return zsb, bcast

                def normalize(head, zsb, bcast, last=False):
                    p, sub = head // 2, head % 2
                    hsl = slice(sub * 64, sub * 64 + 64)
                    zt = zpair[p]
                    # last pair: per-head tiles (rows 0-63 each) so pass2's
                    # first contraction half starts before head 5 normalizes
                    dst = zt[sub][0:64, :] if isinstance(zt, list) else zt[hsl, :]
                    nc.vector.tensor_mul(dst, zsb[:], bcast[:])
                return release, normalize

            def attention(qb, drain=None, late=None, last_pair_drain=None,
                          zpair_override=None):
                nkb = 4 * qb + 4
                dq = list(drain) if drain is not None else []
                iters = [PAIRS * max(nkb - 1, 1), 0]

                def drain_some():
                    if not dq:
                        return
                    n = max(1, -(-len(dq) // max(iters[0] - iters[1], 1)))
                    for _ in range(n):
                        if dq:
                            dq.pop(0)()
                    iters[1] += 1
                zpair = zpair_override or [
                    zp.tile([128, QB], DT, tag=f"zp{p}", name=f"zp{p}_{qb}")
                    for p in range(PAIRS)]
                release, normalize = make_normalize(qb, zpair)
                pending = []
                for p in range(PAIRS):
                    zab = [psA.tile([VO + VH, QB], F32, tag="z", bufs=2,
                                    name=f"zps{qb}_{2 * p + s}") for s in range(2)]

                    def qk(kb):
                        # both heads of the pair, concurrent via PE row groups;
                        # on diagonal blocks only the causally-live columns.
                        o = kb - 4 * qb
                        lo = o * 128 if o > 0 else 0
                        sps = psA.tile([KB, 2 * QB], F32, tag="s", bufs=2,
                                       name=f"sps{qb}_{p}_{kb}")
                        ktt = kt[p][kb // 4]
                        ksl = slice((kb % 4) * KB, (kb % 4 + 1) * KB)
                        nc.tensor.matmul(
                            sps[:, lo:QB], ktt[0:64, ksl],
                            qt[p][qb][0:64, lo:QB],
                            start=True, stop=True, tile_position=(0, 0))
                        nc.tensor.matmul(
                            sps[:, QB + lo:2 * QB], ktt[64:128, ksl],
                            qt[p][qb][64:128, lo:QB],
                            start=True, stop=True, tile_position=(64, 0))
                        return sps, lo

                    def pv(kb, sps, lo):
                        o = kb - 4 * qb
                        pt = wp.tile([KB, 2 * QB], DT, tag="p", bufs=6,
                                     name=f"pt{qb}_{p}_{kb}")
                        if lo == 0:
                            nc.scalar.activation(pt[:], sps[:], Exp,
                                                 scale=float(SCALE))
                        else:
                            s3 = sps[:].rearrange(
                                "p (h s) -> p h s", h=2)[:, :, lo:]
                            p3 = pt[:].rearrange(
                                "p (h s) -> p h s", h=2)[:, :, lo:]
                            nc.scalar.activation(p3, s3, Exp,
                                                 scale=float(SCALE))
                        if o >= 0:  # diagonal: mask the 128-wide tri chunk
                            pm = pt[:].rearrange(
                                "p (h s) -> p h s", h=2)[:, :, o * 128:(o + 1) * 128]
                            nc.gpsimd.tensor_mul(pm, pm, mask3)
                        for s in range(2):
                            nc.tensor.matmul(
                                zab[s][:, lo:QB],
                                vt[kb][:, (2 * p + s) * VW:(2 * p + s + 1) * VW],
                                pt[:, s * QB + lo:(s + 1) * QB],
                                start=(kb == 0), stop=(kb == nkb - 1))

                    prev = qk(0)
                    for kb in range(1, nkb):
                        cur = qk(kb)
                        drain_some()
                        pv(kb - 1, *prev)
                        prev = cur
                        if kb == 2:
                            for args in pending:
                                normalize(*args)
                            pending = []
                            if p == PAIRS - 1 and last_pair_drain is not None:
                                dq.extend(last_pair_drain)
                    pv(nkb - 1, *prev)
                    drain_some()

                    last = (qb == NQB - 1 and p == PAIRS - 1)
                    for s in range(2):
                        head = 2 * p + s
                        zsb, bcast = release(head, zab[s])
                        pending.append((head, zsb, bcast, last))
                for args in pending:
                    normalize(*args)
                while dq:
                    dq.pop(0)()
                if late is not None:
                    for step in late:
                        step()
                return outproj_ops(qb, zpair)

            def outproj_split(qb, zpair):
                """qb=3 variant: p0+p1 partials run early (PE bubbles during
                the last pair), only the short p2 pass waits on the final
                normalize."""
                qsl = slice(qb * QB, (qb + 1) * QB)
                partial = [None] * EC

                def pass1():
                    for e in range(EC):
                        st = {}
                        def mk(e, p, st=st):
                            def step():
                                if p == 0:
                                    st["ps"] = psA.tile(
                                        [128, QB], F32, tag="misc", bufs=2,
                                        name=f"opsa{qb}_{e}")
                                nc.tensor.matmul(
                                    st["ps"][:], wo[p][:, e * 128:(e + 1) * 128],
                                    zpair[p][:], start=(p == 0), stop=(p == 1))
                            return step
                        yield mk(e, 0)
                        yield mk(e, 1)
                        def fin(e, st=st):
                            def step():
                                t = op.tile([128, QB], F32, tag=f"partial{e}",
                                            bufs=1, name=f"partial{qb}_{e}")
                                partial[e] = t
                                nc.vector.tensor_copy(t[:], st["ps"][:])
                            return step
                        yield fin(e)

                def pass2():
                    # split by head: the head-4 half of the contraction runs
                    # while head 5 is still normalizing
                    sts = {}
                    def mka(e):
                        def step():
                            sts[e] = psA.tile([128, QB], F32, tag="misc",
                                              bufs=2, name=f"opsb{qb}_{e}")
                            nc.tensor.matmul(
                                sts[e][:], wo[2][0:64, e * 128:(e + 1) * 128],
                                zpair[2][0][0:64, :], start=True, stop=False,
                                tile_position=(0, 0))
                        return step
                    def mkb(e):
                        def step():
                            nc.tensor.matmul(
                                sts[e][:], wo2hi[0:64, e * 128:(e + 1) * 128],
                                zpair[2][1][0:64, :], start=False, stop=True,
                                tile_position=(0, 0))
                        return step
                    def fin(e):
                        def step():
                            osb = op.tile([128, QB], DT, tag="osb",
                                          name=f"osb{qb}_{e}")
                            nc.vector.scalar_tensor_tensor(
                                osb[:], sts[e][:], bundle[:, 6 + e:7 + e],
                                partial[e][:],
                                op0=mybir.AluOpType.add,
                                op1=mybir.AluOpType.add)
                            nc.sync.dma_start(
                                d_out[e * 128:(e + 1) * 128, qsl], osb[:])
                        return step
                    yield mka(0)
                    yield mka(1)
                    for e in range(EC):
                        yield mkb(e)
                        yield fin(e)
                        if e + 2 < EC:
                            yield mka(e + 2)
                return pass1, pass2

            def outproj_ops(qb, zpair):
                qsl = slice(qb * QB, (qb + 1) * QB)
                for e in range(EC):
                    st = {}
                    def mk(e, p):
                        def step():
                            if p == 0:
                                st["ps"] = psA.tile([128, QB], F32, tag="misc",
                                                    bufs=2, name=f"ops{qb}_{e}")
                            nc.tensor.matmul(
                                st["ps"][:], wo[p][:, e * 128:(e + 1) * 128],
                                zpair[p][:],
                                start=(p == 0), stop=(p == PAIRS - 1))
                        return step
                    for p in range(PAIRS):
                        yield mk(e, p)
                    def fin(e):
                        def step():
                            osb = op.tile([128, QB], DT, tag="osb",
                                          name=f"osb{qb}_{e}")
                            nc.vector.tensor_scalar_add(
                                osb[:], st["ps"][:], bundle[:, 6 + e:7 + e])
                            nc.sync.dma_start(d_out[e * 128:(e + 1) * 128, qsl],
                                              osb[:])
                        return step
                    yield fin(e)

            # startup: K/Q of pair 0 only, then the attention loop starts;
            # the qb0 V chains drain at the FRONT of the w0 queue (one full
            # 7-closure V chain per drain slot beats each pv into the PE
            # FIFO, so no deadlock), followed by the rest of proj(0/1).
            for step in kq_pair_ops(0, 0):
                step()
            w0_drain = (list(v_ops(0))
                        + list(kq_pair_ops(0, 1)) + list(kq_pair_ops(0, 2))
                        + list(kq_ops(1)))
            op0 = list(attention(0, drain=iter(w0_drain), late=v_ops(1)))
            op1 = list(attention(1, drain=iter(kq_ops(2)), late=v_ops(2)))

            # partition-0 copy of wo[2] rows 64-127 so pass2's second half can
            # run in PE row group 0 (serialized with the first half — avoids
            # a concurrent-accumulate drain race into the same PSUM).
            # Emitted here (not at startup) so the DVE queue never head-of-line
            # blocks on the late wo DMA.
            wo2hi = pp.tile([128, E], DT, tag="wo2hi", name="wo2hi")
            nc.vector.tensor_copy(wo2hi[0:64, :], wo_all[64:128, 2 * E:3 * E])

            op2 = list(attention(2, drain=iter(kq_ops(3)), late=v_ops(3)))

            zpair_last = [zp.tile([128, QB], DT, tag=f"zpL{p}",
                                  name=f"zpL{p}") for p in range(2)]
            zpair_last.append(
                [zp.tile([128, QB], DT, tag="zp2h", bufs=2,
                         name=f"zpL2_{s}") for s in range(2)])
            pass1, pass2 = outproj_split(3, zpair_last)
            pass1_steps = list(pass1())
            attention(3, drain=iter(op0 + op1 + op2),
                      last_pair_drain=iter(pass1_steps[:9]),
                      zpair_override=zpair_last)
            for step in pass1_steps[9:]:
                step()
            for step in pass2():
                step()

    nc.compile()
    return nc


def _get_nc():
    if _g["nc"] is None:
        _g["nc"] = _build()
    return _g["nc"]


def _make_in_maps(inputs):
    x = np.asarray(inputs["normalized_resid_pre"], dtype=np.float32)
    W_Q = np.asarray(inputs["W_Q"], dtype=np.float32)
    W_K = np.asarray(inputs["W_K"], dtype=np.float32)
    W_V = np.asarray(inputs["W_V"], dtype=np.float32)
    W_O = np.asarray(inputs["W_O"], dtype=np.float32)
    b_Q = np.asarray(inputs["b_Q"], dtype=np.float32)
    b_K = np.asarray(inputs["b_K"], dtype=np.float32)
    b_V = np.asarray(inputs["b_V"], dtype=np.float32)
    dt = _np_dt()

    # 0/1 keep-mask for the 128-wide diagonal triangle chunk, duplicated for
    # the two heads of a pair: keep when k-within-chunk <= q-within-chunk.
    tri = np.tril(np.ones((KB, KB), dtype=np.float32)).T  # [dk, dq] keep dk<=dq
    mask = np.concatenate([tri, tri], axis=1).astype(dt)  # [128, 256]

    in_maps = []
    for c in range(8):
        b = c // 2
        hs = (c % 2) * HPC
        heads = list(range(hs, hs + HPC))
        def pack(w):
            # [E, C] -> [128, EC*C] with column block e holding rows e*128..
            C = w.shape[1]
            return np.ascontiguousarray(
                w.reshape(EC, 128, C).transpose(1, 0, 2).reshape(128, EC * C))

        wq = np.concatenate(
            [pack(np.concatenate([W_Q[heads[2 * p]], W_Q[heads[2 * p + 1]]], axis=1))
             for p in range(PAIRS)], axis=1)             # [128, 3*768]
        wk = np.concatenate(
            [pack(np.concatenate([W_K[heads[2 * p]], W_K[heads[2 * p + 1]]], axis=1))
             for p in range(PAIRS)], axis=1)
        wv = pack(np.concatenate([W_V[h] for h in heads], axis=1))  # [128, 6*384]
        # wo per pair packed [128, 768]: wo[p, e] = W_O_pair[p-th row, e]
        wo = np.concatenate(
            [np.concatenate([W_O[heads[2 * p]], W_O[heads[2 * p + 1]]], axis=0)
             .reshape(128, E)
             for p in range(PAIRS)], axis=1)             # [128, 3*768]

        # x prepack: tile (quarter, half) = [128, 3*512] with
        # cols c*512+s = xT[(half*3+c)*128+p, quarter*512+s]
        xb = x[b]                                        # [S, E]
        # [quarter, half, 3, 128, 512] -> [128, quarter*half*3*512]
        xr = xb.reshape(NQB, QB, 2, 3, 128).transpose(4, 0, 2, 3, 1)
        xp = np.ascontiguousarray(xr.reshape(128, NQB * 2 * 3 * QB))

        # bias bundle [128, 12]: cols 0-2 bQ pairs, 3-5 bK pairs, 6-11 bO_eff
        bundle = np.zeros((128, 12), dtype=np.float32)
        for p in range(PAIRS):
            bundle[:, p] = np.concatenate(
                [b_Q[heads[2 * p]], b_Q[heads[2 * p + 1]]])
            bundle[:, 3 + p] = np.concatenate(
                [b_K[heads[2 * p]], b_K[heads[2 * p + 1]]])
        # fold b_V into b_O: out += sum_h Wo[h] @ bV[h]  (sum(P)/den == 1)
        # (b_O itself is applied on the host after the gather)
        bo_eff = np.einsum(
            "nhe,nh->e", W_O[heads], b_V[heads]).astype(np.float32)
        bundle[:, 6:12] = bo_eff.reshape(EC, 128).T

        in_maps.append({
            "xp": xp.astype(dt),
            "wq": wq.astype(dt), "wk": wk.astype(dt),
            "wv": wv.astype(dt), "wo": wo.astype(dt),
            "bundle": bundle,
            "mask": mask,
        })
    return in_maps


def _gather(results, b_O):
    out = np.empty((B, S, E), dtype=np.float32)
    for b in range(B):
        acc = results[2 * b]["outT"].astype(np.float32) + \
              results[2 * b + 1]["outT"].astype(np.float32)
        out[b] = acc.T + b_O
    return out


def run(inputs, trace=False):
    """Returns (output, BassKernelResults)."""
    from concourse.bass_utils import run_bass_kernel_spmd

    if trace:
        _install_ntff_shim()
    nc = _get_nc()
    in_maps = _make_in_maps(inputs)
    res = run_bass_kernel_spmd(nc, in_maps, core_ids=list(range(8)), trace=trace)
    b_O = np.asarray(inputs["b_O"], dtype=np.float32)
    return _gather(res.results, b_O), res


def kernel(**inputs):
    out, _ = run(inputs, trace=False)
    return out


def _install_ntff_shim():
    """The agent image's antenv lacks axon_hooks; recreate it so
    run_bass_kernel_spmd(trace=True) can capture NTFF profiles."""
    import types, ctypes, contextlib

    if "antenv.axon_hooks" in sys.modules:
        return
    so_path = "/opt/axon/libaxon_pjrt.so"
    try:
        lib = ctypes.CDLL(so_path)
        lib.axon_start_nrt_profile.argtypes = [ctypes.POINTER(ctypes.c_int64),
                                              ctypes.c_size_t]
        lib.axon_start_nrt_profile.restype = ctypes.c_int64
        lib.axon_stop_nrt_profile.argtypes = [ctypes.c_char_p]
        lib.axon_stop_nrt_profile.restype = ctypes.c_int64
    except (OSError, AttributeError):
        return

    @contextlib.contextmanager
    def _hook(output_dir, device_ids):
        import jax

        jax.devices()
        if device_ids:
            ids = (ctypes.c_int64 * len(device_ids))(*device_ids)
            rc = lib.axon_start_nrt_profile(ids, len(device_ids))
        else:
            rc = lib.axon_start_nrt_profile(None, 0)
        if rc != 0:
            raise RuntimeError(f"axon_start_nrt_profile rc={rc}")
        try:
            yield
        finally:
            n = lib.axon_stop_nrt_profile(str(output_dir).encode())
            print(f"ntff profile: {n} file(s) -> {output_dir}", file=sys.stderr)

    mod = types.ModuleType("antenv.axon_hooks")
    mod.get_axon_ntff_profile_hook = lambda: _hook
    sys.modules["antenv.axon_hooks"] = mod
    # avoid S3 upload attempts from the trace post-processing
    from concourse import bass_utils as bu

    bu.upload_artifacts = lambda tmpdir: f"local:{tmpdir}"


# revision 15
# speedup vs baseline: 1.0344x; 1.0344x over previous
"""Causal multi-head attention (B=4, S=2048, E=768, N=12 heads, H=64) on 8
Trainium2 NeuronCores.

Sharding: core c handles batch c//2 and heads (c%2)*6 .. +6 (tensor parallel
over heads within a batch pair). No collectives: each core emits a partial
out^T = (sum over its 6 heads of z @ W_O) + b_O/2, and the host sums the two
partials per batch and transposes back.

Layout: all device math runs in a transposed layout (seq on the free axis):
  xT [E, S] per batch (host-packed into per-tile contiguous layout)
  Q^T/K^T per head-pair  [128 (2x64h), S] in per-512-column tiles
  V natural [S, 128*6] (col 0 per head is all-ones -> PV matmul row 0
                        accumulates the softmax denominator for free; value
                        columns sit at 64-127 so the z rows of the PV PSUM
                        start at partition 64 — engine partition accesses
                        must be 32-aligned; the ones/dead columns are memset
                        once, V projection writes only cols 64-127 through a
                        strided AP)
  S^T [k, q] scores, both heads of a pair computed concurrently in the PE
  array via tile_position row groups; on diagonal blocks the moving range of
  QK/PV and the exp width are restricted to the causally-live columns and
  only the 128-wide triangle chunk is masked (one [128,2,128] multiply on
  GPSIMD against a host tri-mask); P = exp(scale*S^T); z^T [64, q] is
  normalized straight out of PSUM: 1/den via DVE fast reciprocal on the
  PSUM den row, gpsimd partition_broadcast, one DVE multiply PSUM->SBUF;
  out^T [E, S] accumulated over head pairs (K=128 contraction), written bf16
  and summed/bias'd on the host.

Engine budget: ACT runs ONLY the exp stream (plus the qb=0 K/Q copies that
land in its idle startup window); all other K/Q copy-outs are DVE
tensor_scalar_add with the fused bias, V and out-projection copy-outs are
DVE, diagonal masks and the reciprocal broadcast run on GPSIMD. b_V is
folded into b_O on the host (out += P@(v+bV)/den @ Wo == out + Wo@bV since
sum(P)/den==1) and b_O is applied on the host after the partial-sum gather.

DMAs: every DRAM tensor is host-prepacked to the exact SBUF tile layout so
each transfer is one fully-contiguous descriptor run (3KB+ per partition
line). Startup loads are spread over the gpsimd/scalar/vector/sync queues so
the first K-projection starts ~1.5us in; the scalar queue is idle from the
first exp onward.

Scheduling: window qb drains the K/Q projection chains for qb+1 between the
attention matmuls (V chains run at window end); the output projections are
deferred to the LAST window (qb=3), where the exp stream runs ~20us longer
than the PE work, so they fill PE idle instead of stalling the early
ACT-starved windows.
"""

import sys

sys.path.insert(0, "/opt/trn_rl_repo")

import numpy as np

B, S, E = 4, 2048, 768
N_HEADS, H = 12, 64
HPC = 6           # heads per core
PAIRS = 3         # head pairs per core
EC = E // 128     # 6 e-chunks
QB = 512          # query block (free dim of most matmuls)
NQB = S // QB     # 4
KB = 128          # key sub-block (partition dim of S^T)
SC = S // 128     # 16 s-chunks for V
VW = 128          # V width per head: col 0 = ones (denominator), 64-127 = values
VH = 64           # value columns per head
VO = 64           # value column offset within the per-head block
SCALE = 1.0 / np.sqrt(np.float32(H))

COMPUTE_DT = "bfloat16"

_g = {"nc": None}


def _np_dt():
    if COMPUTE_DT == "bfloat16":
        import ml_dtypes

        return ml_dtypes.bfloat16
    return np.float32


def _build(num_devices=8):
    from concourse import bacc, tile, mybir, library_config

    F32 = mybir.dt.float32
    DT = getattr(mybir.dt, COMPUTE_DT)

    nc = bacc.Bacc("TRN2", target_bir_lowering=False, debug=False,
                   num_devices=num_devices)

    # host-prepacked, per-tile contiguous layouts
    d_x = nc.dram_tensor("xp", [128, NQB * 2 * 3 * QB], DT,
                         kind="ExternalInput").ap()
    d_wq = nc.dram_tensor("wq", [128, PAIRS * E], DT, kind="ExternalInput").ap()
    d_wk = nc.dram_tensor("wk", [128, PAIRS * E], DT, kind="ExternalInput").ap()
    d_wv = nc.dram_tensor("wv", [128, VH * HPC * EC], DT, kind="ExternalInput").ap()
    d_wo = nc.dram_tensor("wo", [128, PAIRS * E], DT, kind="ExternalInput").ap()
    # bundle cols: 0-2 bQ per pair, 3-5 bK per pair, 6-11 effective bO per e
    d_bundle = nc.dram_tensor("bundle", [128, 12], F32, kind="ExternalInput").ap()
    d_mask = nc.dram_tensor("mask", [KB, 2 * KB], DT, kind="ExternalInput").ap()
    d_out = nc.dram_tensor("outT", [E, S], DT, kind="ExternalOutput").ap()

    Exp = mybir.ActivationFunctionType.Exp
    Iden = mybir.ActivationFunctionType.Identity

    with tile.TileContext(nc) as tc:
        with tc.tile_pool(name="persist", bufs=1) as pp, \
             tc.tile_pool(name="work", bufs=4) as wp, \
             tc.tile_pool(name="zsb", bufs=3) as zp, \
             tc.tile_pool(name="outsb", bufs=4) as op, \
             tc.tile_pool(name="psA", bufs=1, space="PSUM") as psA:

            nc.gpsimd.load_library(library_config.proxy)

            # ---- input DMAs --------------------------------------------------
            # First-needed pieces land first on the three DMA-capable queues:
            #   gpsimd: wk p0 | wv e0-2 | wk p1/p2 | wv e3-5
            #   scalar: bundle | wq p0 | wq p1/p2  (idle before the first exp)
            #   sync:   x q0 | mask | x q1..q3 | wo (wo only needed at qb=3)
            wk_all = pp.tile([128, PAIRS * E], DT, tag="wk", name="wk_all")
            wq_all = pp.tile([128, PAIRS * E], DT, tag="wq", name="wq_all")
            wv_all = pp.tile([128, VH * HPC * EC], DT, tag="wv", name="wv_all")
            wo_all = pp.tile([128, PAIRS * E], DT, tag="wo", name="wo_all")
            bundle = pp.tile([128, 12], F32, tag="bundle", name="bundle")
            masksb = pp.tile([KB, 2 * KB], DT, tag="mask", name="masksb")

            nc.gpsimd.dma_start(wk_all[:, 0:E], d_wk[:, 0:E])
            nc.scalar.dma_start(bundle[:], d_bundle[:, :])
            nc.scalar.dma_start(wq_all[:, 0:E], d_wq[:, 0:E])
            nc.gpsimd.dma_start(wv_all[:, 0:3 * VH * HPC],
                                d_wv[:, 0:3 * VH * HPC])
            nc.scalar.dma_start(wq_all[:, E:3 * E], d_wq[:, E:3 * E])
            nc.gpsimd.dma_start(wk_all[:, E:3 * E], d_wk[:, E:3 * E])
            nc.gpsimd.dma_start(wv_all[:, 3 * VH * HPC:], d_wv[:, 3 * VH * HPC:])

            wk = [wk_all[:, p * E:(p + 1) * E] for p in range(PAIRS)]
            wq = [wq_all[:, p * E:(p + 1) * E] for p in range(PAIRS)]
            wo = [wo_all[:, p * E:(p + 1) * E] for p in range(PAIRS)]
            wv = [wv_all[:, e * VH * HPC:(e + 1) * VH * HPC] for e in range(EC)]
            mask3 = masksb[:].rearrange("p (h s) -> p h s", h=2)

            # x: one contiguous DMA per (quarter, half-of-e-chunks) tile.
            xq = [[None, None] for _ in range(4)]

            def x_dma(quarter, half):
                t = pp.tile([128, 3 * QB], DT, tag=f"xq{quarter}_{half}",
                            name=f"xq{quarter}_{half}")
                base = (quarter * 2 + half) * 3 * QB
                nc.sync.dma_start(t[:], d_x[:, base:base + 3 * QB])
                xq[quarter][half] = t

            x_dma(0, 0)
            x_dma(0, 1)
            nc.sync.dma_start(masksb[:], d_mask[:, :])
            for quarter in range(1, 4):
                x_dma(quarter, 0)
                x_dma(quarter, 1)
            nc.sync.dma_start(wo_all[:], d_wo[:, :])

            # HAM warm-up: dummy matmuls during the input-DMA wait so the
            # real stream starts at 2.4GHz instead of the cold 1.2.
            warm = pp.tile([128, QB], DT, tag="warm", name="warm")
            nc.vector.memset(warm[:], 0.0)
            for i in range(6):
                wps = psA.tile([128, QB], F32, tag="misc", bufs=2,
                               name=f"warm{i}")
                nc.tensor.matmul(wps[:], warm[:, 0:128], warm[:],
                                 start=True, stop=True)

            def xchunk(e, sb, lo=0, w=QB):
                # [128, w] slice of e-chunk e, query block sb
                base = (e % 3) * QB + lo
                return xq[sb][e // 3][:, base:base + w]

            kt = [[pp.tile([128, QB], DT, tag=f"kt{p}_{sb}", name=f"kt{p}_{sb}")
                   for sb in range(NQB)] for p in range(PAIRS)]
            qt = [[pp.tile([128, QB], DT, tag=f"qt{p}_{sb}", name=f"qt{p}_{sb}")
                   for sb in range(NQB)] for p in range(PAIRS)]
            vt = [pp.tile([128, VW * HPC], DT, tag=f"vt{s}", name=f"vt{s}")
                  for s in range(SC)]
            # ones columns for the denominator trick: memset only col 0 of
            # each per-head block (cols 1-63 feed PSUM rows that are never
            # read); projections only ever write the 64 value columns.
            for s in range(SC):
                nc.vector.memset(
                    vt[s][:].rearrange("p (h w) -> p h w", w=VW)[:, :, 0:1],
                    1.0)

            def _mk_chain():
                def chain(name, width, lhs_of_e, rhs_of_e, copy_out):
                    st = {}
                    def mk(e):
                        def step():
                            if e == 0:
                                st["ps"] = psA.tile(
                                    [128, width], F32, tag="misc", bufs=2,
                                    name=name)
                            nc.tensor.matmul(st["ps"][:],
                                             lhs_of_e(e), rhs_of_e(e),
                                             start=(e == 0), stop=(e == EC - 1))
                        return step
                    for e in range(EC):
                        yield mk(e)
                    yield lambda: copy_out(st["ps"])
                return chain

            def kq_pair_ops(sb, p, chain=None):
                chain = chain or _mk_chain()
                if sb == 0:
                    # startup window: ACT is otherwise idle here
                    kcopy = lambda ps, p=p, sb=sb: nc.scalar.activation(
                        kt[p][sb][:], ps[:], Iden, bias=bundle[:, 3 + p:4 + p])
                    qcopy = lambda ps, p=p, sb=sb: nc.scalar.activation(
                        qt[p][sb][:], ps[:], Iden, bias=bundle[:, p:p + 1])
                else:
                    kcopy = lambda ps, p=p, sb=sb: nc.vector.tensor_scalar_add(
                        kt[p][sb][:], ps[:], bundle[:, 3 + p:4 + p])
                    qcopy = lambda ps, p=p, sb=sb: nc.vector.tensor_scalar_add(
                        qt[p][sb][:], ps[:], bundle[:, p:p + 1])
                yield from chain(
                    f"kps{p}_{sb}", QB,
                    lambda e, p=p: wk[p][:, e * 128:(e + 1) * 128],
                    lambda e, sb=sb: xchunk(e, sb), kcopy)
                yield from chain(
                    f"qps{p}_{sb}", QB,
                    lambda e, p=p: wq[p][:, e * 128:(e + 1) * 128],
                    lambda e, sb=sb: xchunk(e, sb), qcopy)

            def kq_ops(sb, chain=None):
                for p in range(PAIRS):
                    yield from kq_pair_ops(sb, p, chain)

            def v_chunk_ops(sb, s, chain=None):
                chain = chain or _mk_chain()
                def vcopy(ps, s=s, sb=sb):
                    dst = vt[s][:].rearrange(
                        "p (h w) -> p h w", w=VW)[:, :, VO:VO + VH]
                    srcv = ps[:].rearrange("p (h w) -> p h w", w=VH)
                    if sb == 0:
                        nc.scalar.activation(dst, srcv, Iden)
                    else:
                        nc.vector.tensor_copy(dst, srcv)
                yield from chain(
                    f"vps{s}", VH * HPC,
                    lambda e, sb=sb, s=s: xchunk(e, sb, (s % 4) * 128, 128),
                    lambda e: wv[e], vcopy)

            def v_ops(sb, chain=None):
                for s in range(4 * sb, 4 * sb + 4):
                    yield from v_chunk_ops(sb, s, chain)

            def make_normalize(qb, zpair):
                def release(head, zab):
                    # Emitted right at the pair end so the zab PSUM banks
                    # free before the next pair's first PV (tag z rotates 2
                    # buffers per pair): one f32 copy of the z rows plus the
                    # fast reciprocal of the PSUM den row (partition 0, as
                    # reciprocal_approx_fast requires). The broadcast runs on
                    # gpsimd immediately (nothing queued behind it); the
                    # normalize multiply is deferred so the DVE queue never
                    # head-of-line blocks on the broadcast.
                    zsb = wp.tile([VH, QB], F32, tag="zc", bufs=6,
                                  name=f"zsb{qb}_{head}")
                    nc.vector.tensor_copy(zsb[:], zab[VO:VO + VH, :])
                    recipf = wp.tile([1, QB], F32, tag="recipf", bufs=6,
                                     name=f"recipf{qb}_{head}")
                    nc.vector.reciprocal_approx_fast(recipf[:], zab[0:1, :])
                    bcast = wp.tile([64, QB], F32, tag="bcast", bufs=6,
                                    name=f"bcast{qb}_{head}")
                    nc.gpsimd.partition_broadcast(bcast[:], recipf[:])
                    return zsb, bcast

                def normalize(head, zsb, bcast, last=False):
                    p, sub = head // 2, head % 2
                    hsl = slice(sub * 64, sub * 64 + 64)
                    zt = zpair[p]
                    # last pair: per-head tiles (rows 0-63 each) so pass2's
                    # first contraction half starts before head 5 normalizes
                    dst = zt[sub][0:64, :] if isinstance(zt, list) else zt[hsl, :]
                    nc.vector.tensor_mul(dst, zsb[:], bcast[:])
                return release, normalize

            def attention(qb, drain=None, late=None, last_pair_drain=None,
                          zpair_override=None, pre_pv=None):
                nkb = 4 * qb + 4
                dq = list(drain) if drain is not None else []
                pre_pv = dict(pre_pv) if pre_pv else {}

                def run_pre_pv(kb):
                    for step in pre_pv.pop(kb, ()):
                        step()
                iters = [PAIRS * max(nkb - 1, 1), 0]

                def drain_some():
                    if not dq:
                        return
                    n = max(1, -(-len(dq) // max(iters[0] - iters[1], 1)))
                    for _ in range(n):
                        if dq:
                            dq.pop(0)()
                    iters[1] += 1
                zpair = zpair_override or [
                    zp.tile([128, QB], DT, tag=f"zp{p}", name=f"zp{p}_{qb}")
                    for p in range(PAIRS)]
                release, normalize = make_normalize(qb, zpair)
                pending = []
                for p in range(PAIRS):
                    zab = [psA.tile([VO + VH, QB], F32, tag="z", bufs=2,
                                    name=f"zps{qb}_{2 * p + s}") for s in range(2)]

                    def qk(kb):
                        # both heads of the pair, concurrent via PE row groups;
                        # on diagonal blocks only the causally-live columns.
                        o = kb - 4 * qb
                        lo = o * 128 if o > 0 else 0
                        sps = psA.tile([KB, 2 * QB], F32, tag="s", bufs=2,
                                       name=f"sps{qb}_{p}_{kb}")
                        ktt = kt[p][kb // 4]
                        ksl = slice((kb % 4) * KB, (kb % 4 + 1) * KB)
                        nc.tensor.matmul(
                            sps[:, lo:QB], ktt[0:64, ksl],
                            qt[p][qb][0:64, lo:QB],
                            start=True, stop=True, tile_position=(0, 0))
                        nc.tensor.matmul(
                            sps[:, QB + lo:2 * QB], ktt[64:128, ksl],
                            qt[p][qb][64:128, lo:QB],
                            start=True, stop=True, tile_position=(64, 0))
                        return sps, lo

                    def pv(kb, sps, lo):
                        o = kb - 4 * qb
                        pt = wp.tile([KB, 2 * QB], DT, tag="p", bufs=6,
                                     name=f"pt{qb}_{p}_{kb}")
                        if lo == 0:
                            nc.scalar.activation(pt[:], sps[:], Exp,
                                                 scale=float(SCALE))
                        else:
                            s3 = sps[:].rearrange(
                                "p (h s) -> p h s", h=2)[:, :, lo:]
                            p3 = pt[:].rearrange(
                                "p (h s) -> p h s", h=2)[:, :, lo:]
                            nc.scalar.activation(p3, s3, Exp,
                                                 scale=float(SCALE))
                        if o >= 0:  # diagonal: mask the 128-wide tri chunk
                            pm = pt[:].rearrange(
                                "p (h s) -> p h s", h=2)[:, :, o * 128:(o + 1) * 128]
                            nc.vector.tensor_mul(pm, pm, mask3)
                        for s in range(2):
                            nc.tensor.matmul(
                                zab[s][:, lo:QB],
                                vt[kb][:, (2 * p + s) * VW:(2 * p + s + 1) * VW],
                                pt[:, s * QB + lo:(s + 1) * QB],
                                start=(kb == 0), stop=(kb == nkb - 1))

                    prev = qk(0)
                    for kb in range(1, nkb):
                        cur = qk(kb)
                        run_pre_pv(kb - 1)
                        if kb == 1:
                            drain_some()
                        pv(kb - 1, *prev)
                        if kb > 1:
                            drain_some()
                        prev = cur
                        if kb == 2:
                            for args in pending:
                                normalize(*args)
                            pending = []
                            if p == PAIRS - 1 and last_pair_drain is not None:
                                dq.extend(last_pair_drain)
                    run_pre_pv(nkb - 1)
                    pv(nkb - 1, *prev)
                    drain_some()

                    last = (qb == NQB - 1 and p == PAIRS - 1)
                    for s in range(2):
                        head = 2 * p + s
                        zsb, bcast = release(head, zab[s])
                        pending.append((head, zsb, bcast, last))
                for args in pending:
                    normalize(*args)
                while dq:
                    dq.pop(0)()
                if late is not None:
                    for step in late:
                        step()
                return outproj_ops(qb, zpair)

            def outproj_split(qb, zpair):
                """qb=3 variant: p0+p1 partials run early (PE bubbles during
                the last pair), only the short p2 pass waits on the final
                normalize."""
                qsl = slice(qb * QB, (qb + 1) * QB)
                partial = [None] * EC

                def pass1():
                    for e in range(EC):
                        st = {}
                        def mk(e, p, st=st):
                            def step():
                                if p == 0:
                                    st["ps"] = psA.tile(
                                        [128, QB], F32, tag="misc", bufs=2,
                                        name=f"opsa{qb}_{e}")
                                nc.tensor.matmul(
                                    st["ps"][:], wo[p][:, e * 128:(e + 1) * 128],
                                    zpair[p][:], start=(p == 0), stop=(p == 1))
                            return step
                        yield mk(e, 0)
                        yield mk(e, 1)
                        def fin(e, st=st):
                            def step():
                                t = op.tile([128, QB], F32, tag=f"partial{e}",
                                            bufs=1, name=f"partial{qb}_{e}")
                                partial[e] = t
                                nc.vector.tensor_copy(t[:], st["ps"][:])
                            return step
                        yield fin(e)

                def pass2():
                    # split by head: the head-4 half of the contraction runs
                    # while head 5 is still normalizing
                    sts = {}
                    def mka(e):
                        def step():
                            sts[e] = psA.tile([128, QB], F32, tag="misc",
                                              bufs=2, name=f"opsb{qb}_{e}")
                            nc.tensor.matmul(
                                sts[e][:], wo[2][0:64, e * 128:(e + 1) * 128],
                                zpair[2][0][0:64, :], start=True, stop=False,
                                tile_position=(0, 0))
                        return step
                    def mkb(e):
                        def step():
                            nc.tensor.matmul(
                                sts[e][:], wo2hi[0:64, e * 128:(e + 1) * 128],
                                zpair[2][1][0:64, :], start=False, stop=True,
                                tile_position=(0, 0))
                        return step
                    def fin(e):
                        def step():
                            osb = op.tile([128, QB], DT, tag="osb",
                                          name=f"osb{qb}_{e}")
                            nc.vector.scalar_tensor_tensor(
                                osb[:], sts[e][:], bundle[:, 6 + e:7 + e],
                                partial[e][:],
                                op0=mybir.AluOpType.add,
                                op1=mybir.AluOpType.add)
                            nc.sync.dma_start(
                                d_out[e * 128:(e + 1) * 128, qsl], osb[:])
                        return step
                    yield mka(0)
                    yield mka(1)
                    for e in range(EC):
                        yield mkb(e)
                        yield fin(e)
                        if e + 2 < EC:
                            yield mka(e + 2)
                return pass1, pass2

            def outproj_ops(qb, zpair):
                qsl = slice(qb * QB, (qb + 1) * QB)
                for e in range(EC):
                    st = {}
                    def mk(e, p):
                        def step():
                            if p == 0:
                                st["ps"] = psA.tile([128, QB], F32, tag="misc",
                                                    bufs=2, name=f"ops{qb}_{e}")
                            nc.tensor.matmul(
                                st["ps"][:], wo[p][:, e * 128:(e + 1) * 128],
                                zpair[p][:],
                                start=(p == 0), stop=(p == PAIRS - 1))
                        return step
                    for p in range(PAIRS):
                        yield mk(e, p)
                    def fin(e):
                        def step():
                            osb = op.tile([128, QB], DT, tag="osb",
                                          name=f"osb{qb}_{e}")
                            nc.vector.tensor_scalar_add(
                                osb[:], st["ps"][:], bundle[:, 6 + e:7 + e])
                            nc.sync.dma_start(d_out[e * 128:(e + 1) * 128, qsl],
                                              osb[:])
                        return step
                    yield fin(e)

            # startup: K/Q of pair 0 only, then the attention loop starts;
            # the qb0 V chains drain at the FRONT of the w0 queue (one full
            # 7-closure V chain per drain slot beats each pv into the PE
            # FIFO, so no deadlock), followed by the rest of proj(0/1).
            for step in kq_pair_ops(0, 0):
                step()
            w0_drain = (list(kq_pair_ops(0, 1)) + list(kq_pair_ops(0, 2))
                        + list(kq_ops(1)))
            op0 = list(attention(0, drain=iter(w0_drain), late=v_ops(1),
                                 pre_pv={s: list(v_chunk_ops(0, s))
                                         for s in range(4)}))
            op1 = list(attention(1, drain=iter(kq_ops(2)), late=v_ops(2)))

            # partition-0 copy of wo[2] rows 64-127 so pass2's second half can
            # run in PE row group 0 (serialized with the first half — avoids
            # a concurrent-accumulate drain race into the same PSUM).
            # Emitted here (not at startup) so the DVE queue never head-of-line
            # blocks on the late wo DMA.
            wo2hi = pp.tile([128, E], DT, tag="wo2hi", name="wo2hi")
            nc.vector.tensor_copy(wo2hi[0:64, :], wo_all[64:128, 2 * E:3 * E])

            op2 = list(attention(2, drain=iter(kq_ops(3)), late=v_ops(3)))

            zpair_last = [zp.tile([128, QB], DT, tag=f"zpL{p}",
                                  name=f"zpL{p}") for p in range(2)]
            zpair_last.append(
                [zp.tile([128, QB], DT, tag="zp2h", bufs=2,
                         name=f"zpL2_{s}") for s in range(2)])
            pass1, pass2 = outproj_split(3, zpair_last)
            pass1_steps = list(pass1())
            attention(3, drain=iter(op0 + op1 + op2),
                      last_pair_drain=iter(pass1_steps[:9]),
                      zpair_override=zpair_last)
            for step in pass1_steps[9:]:
                step()
            for step in pass2():
                step()

    nc.compile()
    return nc


def _get_nc():
    if _g["nc"] is None:
        _g["nc"] = _build()
    return _g["nc"]


def _make_in_maps(inputs):
    x = np.asarray(inputs["normalized_resid_pre"], dtype=np.float32)
    W_Q = np.asarray(inputs["W_Q"], dtype=np.float32)
    W_K = np.asarray(inputs["W_K"], dtype=np.float32)
    W_V = np.asarray(inputs["W_V"], dtype=np.float32)
    W_O = np.asarray(inputs["W_O"], dtype=np.float32)
    b_Q = np.asarray(inputs["b_Q"], dtype=np.float32)
    b_K = np.asarray(inputs["b_K"], dtype=np.float32)
    b_V = np.asarray(inputs["b_V"], dtype=np.float32)
    dt = _np_dt()

    # 0/1 keep-mask for the 128-wide diagonal triangle chunk, duplicated for
    # the two heads of a pair: keep when k-within-chunk <= q-within-chunk.
    tri = np.tril(np.ones((KB, KB), dtype=np.float32)).T  # [dk, dq] keep dk<=dq
    mask = np.concatenate([tri, tri], axis=1).astype(dt)  # [128, 256]

    in_maps = []
    for c in range(8):
        b = c // 2
        hs = (c % 2) * HPC
        heads = list(range(hs, hs + HPC))
        def pack(w):
            # [E, C] -> [128, EC*C] with column block e holding rows e*128..
            C = w.shape[1]
            return np.ascontiguousarray(
                w.reshape(EC, 128, C).transpose(1, 0, 2).reshape(128, EC * C))

        wq = np.concatenate(
            [pack(np.concatenate([W_Q[heads[2 * p]], W_Q[heads[2 * p + 1]]], axis=1))
             for p in range(PAIRS)], axis=1)             # [128, 3*768]
        wk = np.concatenate(
            [pack(np.concatenate([W_K[heads[2 * p]], W_K[heads[2 * p + 1]]], axis=1))
             for p in range(PAIRS)], axis=1)
        wv = pack(np.concatenate([W_V[h] for h in heads], axis=1))  # [128, 6*384]
        # wo per pair packed [128, 768]: wo[p, e] = W_O_pair[p-th row, e]
        wo = np.concatenate(
            [np.concatenate([W_O[heads[2 * p]], W_O[heads[2 * p + 1]]], axis=0)
             .reshape(128, E)
             for p in range(PAIRS)], axis=1)             # [128, 3*768]

        # x prepack: tile (quarter, half) = [128, 3*512] with
        # cols c*512+s = xT[(half*3+c)*128+p, quarter*512+s]
        xb = x[b]                                        # [S, E]
        # [quarter, half, 3, 128, 512] -> [128, quarter*half*3*512]
        xr = xb.reshape(NQB, QB, 2, 3, 128).transpose(4, 0, 2, 3, 1)
        xp = np.ascontiguousarray(xr.reshape(128, NQB * 2 * 3 * QB))

        # bias bundle [128, 12]: cols 0-2 bQ pairs, 3-5 bK pairs, 6-11 bO_eff
        bundle = np.zeros((128, 12), dtype=np.float32)
        for p in range(PAIRS):
            bundle[:, p] = np.concatenate(
                [b_Q[heads[2 * p]], b_Q[heads[2 * p + 1]]])
            bundle[:, 3 + p] = np.concatenate(
                [b_K[heads[2 * p]], b_K[heads[2 * p + 1]]])
        # fold b_V into b_O: out += sum_h Wo[h] @ bV[h]  (sum(P)/den == 1)
        # (b_O itself is applied on the host after the gather)
        bo_eff = np.einsum(
            "nhe,nh->e", W_O[heads], b_V[heads]).astype(np.float32)
        bundle[:, 6:12] = bo_eff.reshape(EC, 128).T

        in_maps.append({
            "xp": xp.astype(dt),
            "wq": wq.astype(dt), "wk": wk.astype(dt),
            "wv": wv.astype(dt), "wo": wo.astype(dt),
            "bundle": bundle,
            "mask": mask,
        })
    return in_maps


def _gather(results, b_O):
    out = np.empty((B, S, E), dtype=np.float32)
    for b in range(B):
        acc = results[2 * b]["outT"].astype(np.float32) + \
              results[2 * b + 1]["outT"].astype(np.float32)
        out[b] = acc.T + b_O
    return out


def run(inputs, trace=False):
    """Returns (output, BassKernelResults)."""
    from concourse.bass_utils import run_bass_kernel_spmd

    if trace:
        _install_ntff_shim()
    nc = _get_nc()
    in_maps = _make_in_maps(inputs)
    res = run_bass_kernel_spmd(nc, in_maps, core_ids=list(range(8)), trace=trace)
    b_O = np.asarray(inputs["b_O"], dtype=np.float32)
    return _gather(res.results, b_O), res


def kernel(**inputs):
    out, _ = run(inputs, trace=False)
    return out


def _install_ntff_shim():
    """The agent image's antenv lacks axon_hooks; recreate it so
    run_bass_kernel_spmd(trace=True) can capture NTFF profiles."""
    import types, ctypes, contextlib

    if "antenv.axon_hooks" in sys.modules:
        return
    so_path = "/opt/axon/libaxon_pjrt.so"
    try:
        lib = ctypes.CDLL(so_path)
        lib.axon_start_nrt_profile.argtypes = [ctypes.POINTER(ctypes.c_int64),
                                              ctypes.c_size_t]
        lib.axon_start_nrt_profile.restype = ctypes.c_int64
        lib.axon_stop_nrt_profile.argtypes = [ctypes.c_char_p]
        lib.axon_stop_nrt_profile.restype = ctypes.c_int64
    except (OSError, AttributeError):
        return

    @contextlib.contextmanager
    def _hook(output_dir, device_ids):
        import jax

        jax.devices()
        if device_ids:
            ids = (ctypes.c_int64 * len(device_ids))(*device_ids)
            rc = lib.axon_start_nrt_profile(ids, len(device_ids))
        else:
            rc = lib.axon_start_nrt_profile(None, 0)
        if rc != 0:
            raise RuntimeError(f"axon_start_nrt_profile rc={rc}")
        try:
            yield
        finally:
            n = lib.axon_stop_nrt_profile(str(output_dir).encode())
            print(f"ntff profile: {n} file(s) -> {output_dir}", file=sys.stderr)

    mod = types.ModuleType("antenv.axon_hooks")
    mod.get_axon_ntff_profile_hook = lambda: _hook
    sys.modules["antenv.axon_hooks"] = mod
    # avoid S3 upload attempts from the trace post-processing
    from concourse import bass_utils as bu

    bu.upload_artifacts = lambda tmpdir: f"local:{tmpdir}"


# revision 17
# speedup vs baseline: 1.0471x; 1.0123x over previous
"""Causal multi-head attention (B=4, S=2048, E=768, N=12 heads, H=64) on 8
Trainium2 NeuronCores.

Sharding: core c handles batch c//2 and heads (c%2)*6 .. +6 (tensor parallel
over heads within a batch pair). No collectives: each core emits a partial
out^T = (sum over its 6 heads of z @ W_O) + b_O/2, and the host sums the two
partials per batch and transposes back.

Layout: all device math runs in a transposed layout (seq on the free axis):
  xT [E, S] per batch (host-packed into per-tile contiguous layout)
  Q^T/K^T per head-pair  [128 (2x64h), S] in per-512-column tiles
  V natural [S, 128*6] (col 0 per head is all-ones -> PV matmul row 0
                        accumulates the softmax denominator for free; value
                        columns sit at 64-127 so the z rows of the PV PSUM
                        start at partition 64 — engine partition accesses
                        must be 32-aligned; the ones/dead columns are memset
                        once, V projection writes only cols 64-127 through a
                        strided AP)
  S^T [k, q] scores, both heads of a pair computed concurrently in the PE
  array via tile_position row groups; on diagonal blocks the moving range of
  QK/PV and the exp width are restricted to the causally-live columns and
  only the 128-wide triangle chunk is masked (one [128,2,128] multiply on
  GPSIMD against a host tri-mask); P = exp(scale*S^T); z^T [64, q] is
  normalized straight out of PSUM: 1/den via DVE fast reciprocal on the
  PSUM den row, gpsimd partition_broadcast, one DVE multiply PSUM->SBUF;
  out^T [E, S] accumulated over head pairs (K=128 contraction), written bf16
  and summed/bias'd on the host.

Engine budget: ACT runs ONLY the exp stream (plus the qb=0 K/Q copies that
land in its idle startup window); all other K/Q copy-outs are DVE
tensor_scalar_add with the fused bias, V and out-projection copy-outs are
DVE, diagonal masks and the reciprocal broadcast run on GPSIMD. b_V is
folded into b_O on the host (out += P@(v+bV)/den @ Wo == out + Wo@bV since
sum(P)/den==1) and b_O is applied on the host after the partial-sum gather.

DMAs: every DRAM tensor is host-prepacked to the exact SBUF tile layout so
each transfer is one fully-contiguous descriptor run (3KB+ per partition
line). Startup loads are spread over the gpsimd/scalar/vector/sync queues so
the first K-projection starts ~1.5us in; the scalar queue is idle from the
first exp onward.

Scheduling: window qb drains the K/Q projection chains for qb+1 between the
attention matmuls (V chains run at window end); the output projections are
deferred to the LAST window (qb=3), where the exp stream runs ~20us longer
than the PE work, so they fill PE idle instead of stalling the early
ACT-starved windows.
"""

import sys

sys.path.insert(0, "/opt/trn_rl_repo")

import numpy as np

B, S, E = 4, 2048, 768
N_HEADS, H = 12, 64
HPC = 6           # heads per core
PAIRS = 3         # head pairs per core
EC = E // 128     # 6 e-chunks
QB = 512          # query block (free dim of most matmuls)
NQB = S // QB     # 4
KB = 128          # key sub-block (partition dim of S^T)
SC = S // 128     # 16 s-chunks for V
VW = 128          # V width per head: col 0 = ones (denominator), 64-127 = values
VH = 64           # value columns per head
VO = 64           # value column offset within the per-head block
SCALE = 1.0 / np.sqrt(np.float32(H))

COMPUTE_DT = "bfloat16"

_g = {"nc": None}


def _np_dt():
    if COMPUTE_DT == "bfloat16":
        import ml_dtypes

        return ml_dtypes.bfloat16
    return np.float32


def _build(num_devices=8):
    from concourse import bacc, tile, mybir, library_config

    F32 = mybir.dt.float32
    DT = getattr(mybir.dt, COMPUTE_DT)

    nc = bacc.Bacc("TRN2", target_bir_lowering=False, debug=False,
                   num_devices=num_devices)

    # host-prepacked, per-tile contiguous layouts
    d_x = nc.dram_tensor("xp", [128, NQB * 2 * 3 * QB], DT,
                         kind="ExternalInput").ap()
    d_wq = nc.dram_tensor("wq", [128, PAIRS * E], DT, kind="ExternalInput").ap()
    d_wk = nc.dram_tensor("wk", [128, PAIRS * E], DT, kind="ExternalInput").ap()
    d_wv = nc.dram_tensor("wv", [128, VH * HPC * EC], DT, kind="ExternalInput").ap()
    d_wo = nc.dram_tensor("wo", [128, PAIRS * E], DT, kind="ExternalInput").ap()
    # bundle cols: 0-2 bQ per pair, 3-5 bK per pair, 6-11 effective bO per e
    d_bundle = nc.dram_tensor("bundle", [128, 12], F32, kind="ExternalInput").ap()
    d_mask = nc.dram_tensor("mask", [KB, 2 * KB], DT, kind="ExternalInput").ap()
    d_out = nc.dram_tensor("outT", [E, S], DT, kind="ExternalOutput").ap()

    Exp = mybir.ActivationFunctionType.Exp
    Iden = mybir.ActivationFunctionType.Identity

    with tile.TileContext(nc) as tc:
        with tc.tile_pool(name="persist", bufs=1) as pp, \
             tc.tile_pool(name="work", bufs=4) as wp, \
             tc.tile_pool(name="zsb", bufs=3) as zp, \
             tc.tile_pool(name="outsb", bufs=4) as op, \
             tc.tile_pool(name="psA", bufs=1, space="PSUM") as psA:

            nc.gpsimd.load_library(library_config.proxy)

            # ---- input DMAs --------------------------------------------------
            # First-needed pieces land first on the three DMA-capable queues:
            #   gpsimd: wk p0 | wv e0-2 | wk p1/p2 | wv e3-5
            #   scalar: bundle | wq p0 | wq p1/p2  (idle before the first exp)
            #   sync:   x q0 | mask | x q1..q3 | wo (wo only needed at qb=3)
            wk_all = pp.tile([128, PAIRS * E], DT, tag="wk", name="wk_all")
            wq_all = pp.tile([128, PAIRS * E], DT, tag="wq", name="wq_all")
            wv_all = pp.tile([128, VH * HPC * EC], DT, tag="wv", name="wv_all")
            wo_all = pp.tile([128, PAIRS * E], DT, tag="wo", name="wo_all")
            bundle = pp.tile([128, 12], F32, tag="bundle", name="bundle")
            masksb = pp.tile([KB, 2 * KB], DT, tag="mask", name="masksb")

            nc.gpsimd.dma_start(wk_all[:, 0:E], d_wk[:, 0:E])
            nc.scalar.dma_start(bundle[:], d_bundle[:, :])
            x01t = pp.tile([128, 3 * QB], DT, tag="xq0_1", name="xq0_1")
            nc.scalar.dma_start(x01t[:], d_x[:, 3 * QB:6 * QB])
            nc.scalar.dma_start(wq_all[:, 0:E], d_wq[:, 0:E])
            nc.gpsimd.dma_start(wv_all[:, 0:3 * VH * HPC],
                                d_wv[:, 0:3 * VH * HPC])
            nc.scalar.dma_start(wq_all[:, E:3 * E], d_wq[:, E:3 * E])
            nc.gpsimd.dma_start(wk_all[:, E:3 * E], d_wk[:, E:3 * E])
            nc.gpsimd.dma_start(wv_all[:, 3 * VH * HPC:], d_wv[:, 3 * VH * HPC:])

            wk = [wk_all[:, p * E:(p + 1) * E] for p in range(PAIRS)]
            wq = [wq_all[:, p * E:(p + 1) * E] for p in range(PAIRS)]
            wo = [wo_all[:, p * E:(p + 1) * E] for p in range(PAIRS)]
            wv = [wv_all[:, e * VH * HPC:(e + 1) * VH * HPC] for e in range(EC)]
            mask3 = masksb[:].rearrange("p (h s) -> p h s", h=2)

            # x: one contiguous DMA per (quarter, half-of-e-chunks) tile.
            xq = [[None, None] for _ in range(4)]

            def x_dma(quarter, half):
                t = pp.tile([128, 3 * QB], DT, tag=f"xq{quarter}_{half}",
                            name=f"xq{quarter}_{half}")
                base = (quarter * 2 + half) * 3 * QB
                nc.sync.dma_start(t[:], d_x[:, base:base + 3 * QB])
                xq[quarter][half] = t

            x_dma(0, 0)
            xq[0][1] = x01t
            nc.sync.dma_start(masksb[:], d_mask[:, :])
            for quarter in range(1, 4):
                x_dma(quarter, 0)
                x_dma(quarter, 1)
            nc.sync.dma_start(wo_all[:], d_wo[:, :])

            # HAM warm-up: dummy matmuls during the input-DMA wait so the
            # real stream starts at 2.4GHz instead of the cold 1.2.
            warm = pp.tile([128, QB], DT, tag="warm", name="warm")
            nc.vector.memset(warm[:], 0.0)
            for i in range(6):
                wps = psA.tile([128, QB], F32, tag="misc", bufs=2,
                               name=f"warm{i}")
                nc.tensor.matmul(wps[:], warm[:, 0:128], warm[:],
                                 start=True, stop=True)

            def xchunk(e, sb, lo=0, w=QB):
                # [128, w] slice of e-chunk e, query block sb
                base = (e % 3) * QB + lo
                return xq[sb][e // 3][:, base:base + w]

            kt = [[pp.tile([128, QB], DT, tag=f"kt{p}_{sb}", name=f"kt{p}_{sb}")
                   for sb in range(NQB)] for p in range(PAIRS)]
            qt = [[pp.tile([128, QB], DT, tag=f"qt{p}_{sb}", name=f"qt{p}_{sb}")
                   for sb in range(NQB)] for p in range(PAIRS)]
            vt = [pp.tile([128, VW * HPC], DT, tag=f"vt{s}", name=f"vt{s}")
                  for s in range(SC)]
            # ones columns for the denominator trick: memset only col 0 of
            # each per-head block (cols 1-63 feed PSUM rows that are never
            # read); projections only ever write the 64 value columns.
            for s in range(SC):
                nc.vector.memset(
                    vt[s][:].rearrange("p (h w) -> p h w", w=VW)[:, :, 0:1],
                    1.0)

            def _mk_chain():
                def chain(name, width, lhs_of_e, rhs_of_e, copy_out):
                    st = {}
                    def mk(e):
                        def step():
                            if e == 0:
                                st["ps"] = psA.tile(
                                    [128, width], F32, tag="misc", bufs=2,
                                    name=name)
                            nc.tensor.matmul(st["ps"][:],
                                             lhs_of_e(e), rhs_of_e(e),
                                             start=(e == 0), stop=(e == EC - 1))
                        return step
                    for e in range(EC):
                        yield mk(e)
                    yield lambda: copy_out(st["ps"])
                return chain

            def kq_pair_ops(sb, p, chain=None):
                chain = chain or _mk_chain()
                if sb == 0:
                    # startup window: ACT is otherwise idle here
                    kcopy = lambda ps, p=p, sb=sb: nc.scalar.activation(
                        kt[p][sb][:], ps[:], Iden, bias=bundle[:, 3 + p:4 + p])
                    qcopy = lambda ps, p=p, sb=sb: nc.scalar.activation(
                        qt[p][sb][:], ps[:], Iden, bias=bundle[:, p:p + 1])
                else:
                    kcopy = lambda ps, p=p, sb=sb: nc.vector.tensor_scalar_add(
                        kt[p][sb][:], ps[:], bundle[:, 3 + p:4 + p])
                    qcopy = lambda ps, p=p, sb=sb: nc.vector.tensor_scalar_add(
                        qt[p][sb][:], ps[:], bundle[:, p:p + 1])
                yield from chain(
                    f"kps{p}_{sb}", QB,
                    lambda e, p=p: wk[p][:, e * 128:(e + 1) * 128],
                    lambda e, sb=sb: xchunk(e, sb), kcopy)
                yield from chain(
                    f"qps{p}_{sb}", QB,
                    lambda e, p=p: wq[p][:, e * 128:(e + 1) * 128],
                    lambda e, sb=sb: xchunk(e, sb), qcopy)

            def kq_ops(sb, chain=None):
                for p in range(PAIRS):
                    yield from kq_pair_ops(sb, p, chain)

            def v_chunk_ops(sb, s, chain=None):
                chain = chain or _mk_chain()
                def vcopy(ps, s=s, sb=sb):
                    dst = vt[s][:].rearrange(
                        "p (h w) -> p h w", w=VW)[:, :, VO:VO + VH]
                    srcv = ps[:].rearrange("p (h w) -> p h w", w=VH)
                    if sb == 0:
                        nc.scalar.activation(dst, srcv, Iden)
                    else:
                        nc.vector.tensor_copy(dst, srcv)
                yield from chain(
                    f"vps{s}", VH * HPC,
                    lambda e, sb=sb, s=s: xchunk(e, sb, (s % 4) * 128, 128),
                    lambda e: wv[e], vcopy)

            def v_ops(sb, chain=None):
                for s in range(4 * sb, 4 * sb + 4):
                    yield from v_chunk_ops(sb, s, chain)

            def make_normalize(qb, zpair):
                def release(head, zab, last=False):
                    # Emitted right at the pair end so the zab PSUM banks
                    # free before the next pair's first PV (tag z rotates 2
                    # buffers per pair): one f32 copy of the z rows plus the
                    # fast reciprocal of the PSUM den row (partition 0, as
                    # reciprocal_approx_fast requires). The broadcast runs on
                    # gpsimd immediately (nothing queued behind it); the
                    # normalize multiply is deferred so the DVE queue never
                    # head-of-line blocks on the broadcast.
                    if last:
                        zsb = zab[VO:VO + VH, :]
                    else:
                        zt = wp.tile([VH, QB], F32, tag="zc", bufs=6,
                                     name=f"zsb{qb}_{head}")
                        nc.vector.tensor_copy(zt[:], zab[VO:VO + VH, :])
                        zsb = zt[:]
                    recipf = wp.tile([1, QB], F32, tag="recipf", bufs=6,
                                     name=f"recipf{qb}_{head}")
                    nc.vector.reciprocal_approx_fast(recipf[:], zab[0:1, :])
                    bcast = wp.tile([64, QB], F32, tag="bcast", bufs=6,
                                    name=f"bcast{qb}_{head}")
                    nc.gpsimd.partition_broadcast(bcast[:], recipf[:])
                    return zsb, bcast

                def normalize(head, zsb, bcast, last=False):
                    p, sub = head // 2, head % 2
                    hsl = slice(sub * 64, sub * 64 + 64)
                    zt = zpair[p]
                    # last pair: per-head tiles (rows 0-63 each) so pass2's
                    # first contraction half starts before head 5 normalizes
                    dst = zt[sub][0:64, :] if isinstance(zt, list) else zt[hsl, :]
                    nc.vector.tensor_mul(dst, zsb, bcast[:])
                return release, normalize

            def attention(qb, drain=None, late=None, last_pair_drain=None,
                          zpair_override=None, pre_pv=None):
                nkb = 4 * qb + 4
                dq = list(drain) if drain is not None else []
                pre_pv = dict(pre_pv) if pre_pv else {}

                def run_pre_pv(kb):
                    for step in pre_pv.pop(kb, ()):
                        step()
                iters = [PAIRS * max(nkb - 1, 1), 0]

                def drain_some():
                    if not dq:
                        return
                    n = max(1, -(-len(dq) // max(iters[0] - iters[1], 1)))
                    for _ in range(n):
                        if dq:
                            dq.pop(0)()
                    iters[1] += 1
                zpair = zpair_override or [
                    zp.tile([128, QB], DT, tag=f"zp{p}", name=f"zp{p}_{qb}")
                    for p in range(PAIRS)]
                release, normalize = make_normalize(qb, zpair)
                pending = []
                for p in range(PAIRS):
                    zab = [psA.tile([VO + VH, QB], F32, tag="z", bufs=2,
                                    name=f"zps{qb}_{2 * p + s}") for s in range(2)]

                    def qk(kb):
                        # both heads of the pair, concurrent via PE row groups;
                        # on diagonal blocks only the causally-live columns.
                        o = kb - 4 * qb
                        lo = o * 128 if o > 0 else 0
                        sps = psA.tile([KB, 2 * QB], F32, tag="s", bufs=2,
                                       name=f"sps{qb}_{p}_{kb}")
                        ktt = kt[p][kb // 4]
                        ksl = slice((kb % 4) * KB, (kb % 4 + 1) * KB)
                        nc.tensor.matmul(
                            sps[:, lo:QB], ktt[0:64, ksl],
                            qt[p][qb][0:64, lo:QB],
                            start=True, stop=True, tile_position=(0, 0))
                        nc.tensor.matmul(
                            sps[:, QB + lo:2 * QB], ktt[64:128, ksl],
                            qt[p][qb][64:128, lo:QB],
                            start=True, stop=True, tile_position=(64, 0))
                        return sps, lo

                    def pv(kb, sps, lo):
                        o = kb - 4 * qb
                        pt = wp.tile([KB, 2 * QB], DT, tag="p", bufs=6,
                                     name=f"pt{qb}_{p}_{kb}")
                        if lo == 0:
                            nc.scalar.activation(pt[:], sps[:], Exp,
                                                 scale=float(SCALE))
                        else:
                            s3 = sps[:].rearrange(
                                "p (h s) -> p h s", h=2)[:, :, lo:]
                            p3 = pt[:].rearrange(
                                "p (h s) -> p h s", h=2)[:, :, lo:]
                            nc.scalar.activation(p3, s3, Exp,
                                                 scale=float(SCALE))
                        if o >= 0:  # diagonal: mask the 128-wide tri chunk
                            pm = pt[:].rearrange(
                                "p (h s) -> p h s", h=2)[:, :, o * 128:(o + 1) * 128]
                            nc.vector.tensor_mul(pm, pm, mask3)
                        for s in range(2):
                            nc.tensor.matmul(
                                zab[s][:, lo:QB],
                                vt[kb][:, (2 * p + s) * VW:(2 * p + s + 1) * VW],
                                pt[:, s * QB + lo:(s + 1) * QB],
                                start=(kb == 0), stop=(kb == nkb - 1))

                    prev = qk(0)
                    for kb in range(1, nkb):
                        cur = qk(kb)
                        run_pre_pv(kb - 1)
                        drain_some()
                        pv(kb - 1, *prev)
                        prev = cur
                        if kb == 2:
                            for args in pending:
                                normalize(*args)
                            pending = []
                            if p == PAIRS - 1 and last_pair_drain is not None:
                                dq.extend(last_pair_drain)
                    run_pre_pv(nkb - 1)
                    pv(nkb - 1, *prev)
                    drain_some()

                    last = (qb == NQB - 1 and p == PAIRS - 1)
                    for s in range(2):
                        head = 2 * p + s
                        zsb, bcast = release(head, zab[s], last)
                        pending.append((head, zsb, bcast, last))
                for args in pending:
                    normalize(*args)
                while dq:
                    dq.pop(0)()
                if late is not None:
                    for step in late:
                        step()
                return outproj_ops(qb, zpair)

            def outproj_split(qb, zpair):
                """qb=3 variant: p0+p1 partials run early (PE bubbles during
                the last pair), only the short p2 pass waits on the final
                normalize."""
                qsl = slice(qb * QB, (qb + 1) * QB)
                partial = [None] * EC

                def pass1():
                    for e in range(EC):
                        st = {}
                        def mk(e, p, st=st):
                            def step():
                                if p == 0:
                                    st["ps"] = psA.tile(
                                        [128, QB], F32, tag="misc", bufs=2,
                                        name=f"opsa{qb}_{e}")
                                nc.tensor.matmul(
                                    st["ps"][:], wo[p][:, e * 128:(e + 1) * 128],
                                    zpair[p][:], start=(p == 0), stop=(p == 1))
                            return step
                        yield mk(e, 0)
                        yield mk(e, 1)
                        def fin(e, st=st):
                            def step():
                                t = op.tile([128, QB], F32, tag=f"partial{e}",
                                            bufs=1, name=f"partial{qb}_{e}")
                                partial[e] = t
                                nc.vector.tensor_scalar_add(
                                    t[:], st["ps"][:], bundle[:, 6 + e:7 + e])
                            return step
                        yield fin(e)

                def pass2():
                    # split by head: the head-4 half of the contraction runs
                    # while head 5 is still normalizing
                    sts = {}
                    def mka(e):
                        def step():
                            sts[e] = psA.tile([128, QB], F32, tag="misc",
                                              bufs=2, name=f"opsb{qb}_{e}")
                            nc.tensor.matmul(
                                sts[e][:], wo[2][0:64, e * 128:(e + 1) * 128],
                                zpair[2][0][0:64, :], start=True, stop=False,
                                tile_position=(0, 0))
                        return step
                    def mkb(e):
                        def step():
                            nc.tensor.matmul(
                                sts[e][:], wo2hi[0:64, e * 128:(e + 1) * 128],
                                zpair[2][1][0:64, :], start=False, stop=True,
                                tile_position=(0, 0))
                        return step
                    def fin(e):
                        def step():
                            osb = op.tile([128, QB], DT, tag="osb",
                                          name=f"osb{qb}_{e}")
                            nc.vector.tensor_add(osb[:], sts[e][:],
                                                 partial[e][:])
                            nc.sync.dma_start(
                                d_out[e * 128:(e + 1) * 128, qsl], osb[:])
                        return step
                    yield mka(0)
                    yield mka(1)
                    for e in range(EC):
                        yield mkb(e)
                        yield fin(e)
                        if e + 2 < EC:
                            yield mka(e + 2)
                return pass1, pass2

            def outproj_ops(qb, zpair):
                qsl = slice(qb * QB, (qb + 1) * QB)
                for e in range(EC):
                    st = {}
                    def mk(e, p):
                        def step():
                            if p == 0:
                                st["ps"] = psA.tile([128, QB], F32, tag="misc",
                                                    bufs=2, name=f"ops{qb}_{e}")
                            nc.tensor.matmul(
                                st["ps"][:], wo[p][:, e * 128:(e + 1) * 128],
                                zpair[p][:],
                                start=(p == 0), stop=(p == PAIRS - 1))
                        return step
                    for p in range(PAIRS):
                        yield mk(e, p)
                    def fin(e):
                        def step():
                            osb = op.tile([128, QB], DT, tag="osb",
                                          name=f"osb{qb}_{e}")
                            nc.vector.tensor_scalar_add(
                                osb[:], st["ps"][:], bundle[:, 6 + e:7 + e])
                            nc.sync.dma_start(d_out[e * 128:(e + 1) * 128, qsl],
                                              osb[:])
                        return step
                    yield fin(e)

            # startup: K/Q of pair 0 only, then the attention loop starts;
            # the qb0 V chains drain at the FRONT of the w0 queue (one full
            # 7-closure V chain per drain slot beats each pv into the PE
            # FIFO, so no deadlock), followed by the rest of proj(0/1).
            for step in kq_pair_ops(0, 0):
                step()
            w0_drain = (list(kq_pair_ops(0, 1)) + list(kq_pair_ops(0, 2))
                        + list(kq_ops(1)))
            op0 = list(attention(0, drain=iter(w0_drain), late=v_ops(1),
                                 pre_pv={s: list(v_chunk_ops(0, s))
                                         for s in range(4)}))
            op1 = list(attention(1, drain=iter(kq_ops(2)), late=v_ops(2)))

            # partition-0 copy of wo[2] rows 64-127 so pass2's second half can
            # run in PE row group 0 (serialized with the first half — avoids
            # a concurrent-accumulate drain race into the same PSUM).
            # Emitted here (not at startup) so the DVE queue never head-of-line
            # blocks on the late wo DMA.
            wo2hi = pp.tile([128, E], DT, tag="wo2hi", name="wo2hi")
            nc.vector.tensor_copy(wo2hi[0:64, :], wo_all[64:128, 2 * E:3 * E])

            op2 = list(attention(2, drain=iter(kq_ops(3)), late=v_ops(3)))

            zpair_last = [zp.tile([128, QB], DT, tag=f"zpL{p}",
                                  name=f"zpL{p}") for p in range(2)]
            zpair_last.append(
                [zp.tile([128, QB], DT, tag="zp2h", bufs=2,
                         name=f"zpL2_{s}") for s in range(2)])
            pass1, pass2 = outproj_split(3, zpair_last)
            pass1_steps = list(pass1())
            attention(3, drain=iter(op0 + op1 + op2),
                      last_pair_drain=iter(pass1_steps[:9]),
                      zpair_override=zpair_last)
            for step in pass1_steps[9:]:
                step()
            for step in pass2():
                step()

    nc.compile()
    return nc


def _get_nc():
    if _g["nc"] is None:
        _g["nc"] = _build()
    return _g["nc"]


def _make_in_maps(inputs):
    x = np.asarray(inputs["normalized_resid_pre"], dtype=np.float32)
    W_Q = np.asarray(inputs["W_Q"], dtype=np.float32)
    W_K = np.asarray(inputs["W_K"], dtype=np.float32)
    W_V = np.asarray(inputs["W_V"], dtype=np.float32)
    W_O = np.asarray(inputs["W_O"], dtype=np.float32)
    b_Q = np.asarray(inputs["b_Q"], dtype=np.float32)
    b_K = np.asarray(inputs["b_K"], dtype=np.float32)
    b_V = np.asarray(inputs["b_V"], dtype=np.float32)
    dt = _np_dt()

    # 0/1 keep-mask for the 128-wide diagonal triangle chunk, duplicated for
    # the two heads of a pair: keep when k-within-chunk <= q-within-chunk.
    tri = np.tril(np.ones((KB, KB), dtype=np.float32)).T  # [dk, dq] keep dk<=dq
    mask = np.concatenate([tri, tri], axis=1).astype(dt)  # [128, 256]

    in_maps = []
    for c in range(8):
        b = c // 2
        hs = (c % 2) * HPC
        heads = list(range(hs, hs + HPC))
        def pack(w):
            # [E, C] -> [128, EC*C] with column block e holding rows e*128..
            C = w.shape[1]
            return np.ascontiguousarray(
                w.reshape(EC, 128, C).transpose(1, 0, 2).reshape(128, EC * C))

        wq = np.concatenate(
            [pack(np.concatenate([W_Q[heads[2 * p]], W_Q[heads[2 * p + 1]]], axis=1))
             for p in range(PAIRS)], axis=1)             # [128, 3*768]
        wk = np.concatenate(
            [pack(np.concatenate([W_K[heads[2 * p]], W_K[heads[2 * p + 1]]], axis=1))
             for p in range(PAIRS)], axis=1)
        wv = pack(np.concatenate([W_V[h] for h in heads], axis=1))  # [128, 6*384]
        # wo per pair packed [128, 768]: wo[p, e] = W_O_pair[p-th row, e]
        wo = np.concatenate(
            [np.concatenate([W_O[heads[2 * p]], W_O[heads[2 * p + 1]]], axis=0)
             .reshape(128, E)
             for p in range(PAIRS)], axis=1)             # [128, 3*768]

        # x prepack: tile (quarter, half) = [128, 3*512] with
        # cols c*512+s = xT[(half*3+c)*128+p, quarter*512+s]
        xb = x[b]                                        # [S, E]
        # [quarter, half, 3, 128, 512] -> [128, quarter*half*3*512]
        xr = xb.reshape(NQB, QB, 2, 3, 128).transpose(4, 0, 2, 3, 1)
        xp = np.ascontiguousarray(xr.reshape(128, NQB * 2 * 3 * QB))

        # bias bundle [128, 12]: cols 0-2 bQ pairs, 3-5 bK pairs, 6-11 bO_eff
        bundle = np.zeros((128, 12), dtype=np.float32)
        for p in range(PAIRS):
            bundle[:, p] = np.concatenate(
                [b_Q[heads[2 * p]], b_Q[heads[2 * p + 1]]])
            bundle[:, 3 + p] = np.concatenate(
                [b_K[heads[2 * p]], b_K[heads[2 * p + 1]]])
        # fold b_V into b_O: out += sum_h Wo[h] @ bV[h]  (sum(P)/den == 1)
        # (b_O itself is applied on the host after the gather)
        bo_eff = np.einsum(
            "nhe,nh->e", W_O[heads], b_V[heads]).astype(np.float32)
        bundle[:, 6:12] = bo_eff.reshape(EC, 128).T

        in_maps.append({
            "xp": xp.astype(dt),
            "wq": wq.astype(dt), "wk": wk.astype(dt),
            "wv": wv.astype(dt), "wo": wo.astype(dt),
            "bundle": bundle,
            "mask": mask,
        })
    return in_maps


def _gather(results, b_O):
    out = np.empty((B, S, E), dtype=np.float32)
    for b in range(B):
        acc = results[2 * b]["outT"].astype(np.float32) + \
              results[2 * b + 1]["outT"].astype(np.float32)
        out[b] = acc.T + b_O
    return out


def run(inputs, trace=False):
    """Returns (output, BassKernelResults)."""
    from concourse.bass_utils import run_bass_kernel_spmd

    if trace:
        _install_ntff_shim()
    nc = _get_nc()
    in_maps = _make_in_maps(inputs)
    res = run_bass_kernel_spmd(nc, in_maps, core_ids=list(range(8)), trace=trace)
    b_O = np.asarray(inputs["b_O"], dtype=np.float32)
    return _gather(res.results, b_O), res


def kernel(**inputs):
    out, _ = run(inputs, trace=False)
    return out


def _install_ntff_shim():
    """The agent image's antenv lacks axon_hooks; recreate it so
    run_bass_kernel_spmd(trace=True) can capture NTFF profiles."""
    import types, ctypes, contextlib

    if "antenv.axon_hooks" in sys.modules:
        return
    so_path = "/opt/axon/libaxon_pjrt.so"
    try:
        lib = ctypes.CDLL(so_path)
        lib.axon_start_nrt_profile.argtypes = [ctypes.POINTER(ctypes.c_int64),
                                              ctypes.c_size_t]
        lib.axon_start_nrt_profile.restype = ctypes.c_int64
        lib.axon_stop_nrt_profile.argtypes = [ctypes.c_char_p]
        lib.axon_stop_nrt_profile.restype = ctypes.c_int64
    except (OSError, AttributeError):
        return

    @contextlib.contextmanager
    def _hook(output_dir, device_ids):
        import jax

        jax.devices()
        if device_ids:
            ids = (ctypes.c_int64 * len(device_ids))(*device_ids)
            rc = lib.axon_start_nrt_profile(ids, len(device_ids))
        else:
            rc = lib.axon_start_nrt_profile(None, 0)
        if rc != 0:
            raise RuntimeError(f"axon_start_nrt_profile rc={rc}")
        try:
            yield
        finally:
            n = lib.axon_stop_nrt_profile(str(output_dir).encode())
            print(f"ntff profile: {n} file(s) -> {output_dir}", file=sys.stderr)

    mod = types.ModuleType("antenv.axon_hooks")
    mod.get_axon_ntff_profile_hook = lambda: _hook
    sys.modules["antenv.axon_hooks"] = mod
    # avoid S3 upload attempts from the trace post-processing
    from concourse import bass_utils as bu

    bu.upload_artifacts = lambda tmpdir: f"local:{tmpdir}"


# revision 18
# speedup vs baseline: 1.1066x; 1.0568x over previous
"""Causal multi-head attention (B=4, S=2048, E=768, N=12 heads, H=64) on 8
Trainium2 NeuronCores.

Sharding: core c handles batch c//2 and heads (c%2)*6 .. +6 (tensor parallel
over heads within a batch pair). No collectives: each core emits a partial
out^T = (sum over its 6 heads of z @ W_O) + b_O/2, and the host sums the two
partials per batch and transposes back.

Layout: all device math runs in a transposed layout (seq on the free axis):
  xT [E, S] per batch (host-packed into per-tile contiguous layout)
  Q^T/K^T per head-pair  [128 (2x64h), S] in per-512-column tiles
  V natural [S, 128*6] (col 0 per head is all-ones -> PV matmul row 0
                        accumulates the softmax denominator for free; value
                        columns sit at 64-127 so the z rows of the PV PSUM
                        start at partition 64 — engine partition accesses
                        must be 32-aligned; the ones/dead columns are memset
                        once, V projection writes only cols 64-127 through a
                        strided AP)
  S^T [k, q] scores, both heads of a pair computed concurrently in the PE
  array via tile_position row groups; on diagonal blocks the moving range of
  QK/PV and the exp width are restricted to the causally-live columns and
  only the 128-wide triangle chunk is masked (one [128,2,128] multiply on
  GPSIMD against a host tri-mask); P = exp(scale*S^T); z^T [64, q] is
  normalized straight out of PSUM: 1/den via DVE fast reciprocal on the
  PSUM den row, gpsimd partition_broadcast, one DVE multiply PSUM->SBUF;
  out^T [E, S] accumulated over head pairs (K=128 contraction), written bf16
  and summed/bias'd on the host.

Engine budget: ACT runs ONLY the exp stream (plus the qb=0 K/Q copies that
land in its idle startup window); all other K/Q copy-outs are DVE
tensor_scalar_add with the fused bias, V and out-projection copy-outs are
DVE, diagonal masks and the reciprocal broadcast run on GPSIMD. b_V is
folded into b_O on the host (out += P@(v+bV)/den @ Wo == out + Wo@bV since
sum(P)/den==1) and b_O is applied on the host after the partial-sum gather.

DMAs: every DRAM tensor is host-prepacked to the exact SBUF tile layout so
each transfer is one fully-contiguous descriptor run (3KB+ per partition
line). Startup loads are spread over the gpsimd/scalar/vector/sync queues so
the first K-projection starts ~1.5us in; the scalar queue is idle from the
first exp onward.

Scheduling: window qb drains the K/Q projection chains for qb+1 between the
attention matmuls (V chains run at window end); the output projections are
deferred to the LAST window (qb=3), where the exp stream runs ~20us longer
than the PE work, so they fill PE idle instead of stalling the early
ACT-starved windows.
"""

import sys

sys.path.insert(0, "/opt/trn_rl_repo")

import numpy as np

B, S, E = 4, 2048, 768
N_HEADS, H = 12, 64
HPC = 6           # heads per core
PAIRS = 3         # head pairs per core
EC = E // 128     # 6 e-chunks
QB = 512          # query block (free dim of most matmuls)
NQB = S // QB     # 4
KB = 128          # key sub-block (partition dim of S^T)
SC = S // 128     # 16 s-chunks for V
VW = 128          # V width per head: col 0 = ones (denominator), 64-127 = values
VH = 64           # value columns per head
VO = 64           # value column offset within the per-head block
SCALE = 1.0 / np.sqrt(np.float32(H))

COMPUTE_DT = "bfloat16"

_g = {"nc": None}


def _np_dt():
    if COMPUTE_DT == "bfloat16":
        import ml_dtypes

        return ml_dtypes.bfloat16
    return np.float32


def _build(num_devices=8):
    from concourse import bacc, tile, mybir, library_config

    F32 = mybir.dt.float32
    DT = getattr(mybir.dt, COMPUTE_DT)

    nc = bacc.Bacc("TRN2", target_bir_lowering=False, debug=False,
                   num_devices=num_devices)

    # host-prepacked, per-tile contiguous layouts
    d_x = nc.dram_tensor("xp", [128, NQB * 2 * 3 * QB], DT,
                         kind="ExternalInput").ap()
    d_wq = nc.dram_tensor("wq", [128, PAIRS * E], DT, kind="ExternalInput").ap()
    d_wk = nc.dram_tensor("wk", [128, PAIRS * E], DT, kind="ExternalInput").ap()
    d_wv = nc.dram_tensor("wv", [128, VH * HPC * EC], DT, kind="ExternalInput").ap()
    d_wo = nc.dram_tensor("wo", [128, PAIRS * E], DT, kind="ExternalInput").ap()
    # bundle cols: 0-2 bQ per pair, 3-5 bK per pair, 6-11 effective bO per e
    d_bundle = nc.dram_tensor("bundle", [128, 12], F32, kind="ExternalInput").ap()
    d_mask = nc.dram_tensor("mask", [KB, 2 * KB], DT, kind="ExternalInput").ap()
    d_out = nc.dram_tensor("outT", [E, S], DT, kind="ExternalOutput").ap()

    Exp = mybir.ActivationFunctionType.Exp
    Iden = mybir.ActivationFunctionType.Identity

    with tile.TileContext(nc) as tc:
        with tc.tile_pool(name="persist", bufs=1) as pp, \
             tc.tile_pool(name="work", bufs=4) as wp, \
             tc.tile_pool(name="zsb", bufs=3) as zp, \
             tc.tile_pool(name="outsb", bufs=4) as op, \
             tc.tile_pool(name="psA", bufs=1, space="PSUM") as psA:

            # ---- input DMAs --------------------------------------------------
            # First-needed pieces land first on the three DMA-capable queues:
            #   gpsimd: wk p0 | wv e0-2 | wk p1/p2 | wv e3-5
            #   scalar: bundle | wq p0 | wq p1/p2  (idle before the first exp)
            #   sync:   x q0 | mask | x q1..q3 | wo (wo only needed at qb=3)
            wk_all = pp.tile([128, PAIRS * E], DT, tag="wk", name="wk_all")
            wq_all = pp.tile([128, PAIRS * E], DT, tag="wq", name="wq_all")
            wv_all = pp.tile([128, VH * HPC * EC], DT, tag="wv", name="wv_all")
            wo_all = pp.tile([128, PAIRS * E], DT, tag="wo", name="wo_all")
            bundle = pp.tile([128, 12], F32, tag="bundle", name="bundle")
            masksb = pp.tile([KB, 2 * KB], DT, tag="mask", name="masksb")

            nc.gpsimd.dma_start(wk_all[:, 0:E], d_wk[:, 0:E])
            nc.scalar.dma_start(bundle[:], d_bundle[:, :])
            x01t = pp.tile([128, 3 * QB], DT, tag="xq0_1", name="xq0_1")
            nc.scalar.dma_start(x01t[:], d_x[:, 3 * QB:6 * QB])
            nc.scalar.dma_start(wq_all[:, 0:E], d_wq[:, 0:E])
            nc.gpsimd.dma_start(wv_all[:, 0:3 * VH * HPC],
                                d_wv[:, 0:3 * VH * HPC])
            nc.scalar.dma_start(wq_all[:, E:3 * E], d_wq[:, E:3 * E])
            nc.gpsimd.dma_start(wk_all[:, E:3 * E], d_wk[:, E:3 * E])
            nc.gpsimd.dma_start(wv_all[:, 3 * VH * HPC:], d_wv[:, 3 * VH * HPC:])

            wk = [wk_all[:, p * E:(p + 1) * E] for p in range(PAIRS)]
            wq = [wq_all[:, p * E:(p + 1) * E] for p in range(PAIRS)]
            wo = [wo_all[:, p * E:(p + 1) * E] for p in range(PAIRS)]
            wv = [wv_all[:, e * VH * HPC:(e + 1) * VH * HPC] for e in range(EC)]
            mask3 = masksb[:].rearrange("p (h s) -> p h s", h=2)

            # x: one contiguous DMA per (quarter, half-of-e-chunks) tile.
            xq = [[None, None] for _ in range(4)]

            def x_dma(quarter, half):
                t = pp.tile([128, 3 * QB], DT, tag=f"xq{quarter}_{half}",
                            name=f"xq{quarter}_{half}")
                base = (quarter * 2 + half) * 3 * QB
                nc.sync.dma_start(t[:], d_x[:, base:base + 3 * QB])
                xq[quarter][half] = t

            x_dma(0, 0)
            xq[0][1] = x01t
            nc.sync.dma_start(masksb[:], d_mask[:, :])
            for quarter in range(1, 4):
                x_dma(quarter, 0)
                x_dma(quarter, 1)
            nc.sync.dma_start(wo_all[:], d_wo[:, :])

            # HAM warm-up: dummy matmuls during the input-DMA wait so the
            # real stream starts at 2.4GHz instead of the cold 1.2.
            warm = pp.tile([128, QB], DT, tag="warm", name="warm")
            nc.vector.memset(warm[:], 0.0)
            for i in range(6):
                wps = psA.tile([128, QB], F32, tag="misc", bufs=2,
                               name=f"warm{i}")
                nc.tensor.matmul(wps[:], warm[:, 0:128], warm[:],
                                 start=True, stop=True)

            def xchunk(e, sb, lo=0, w=QB):
                # [128, w] slice of e-chunk e, query block sb
                base = (e % 3) * QB + lo
                return xq[sb][e // 3][:, base:base + w]

            kt = [[pp.tile([128, QB], DT, tag=f"kt{p}_{sb}", name=f"kt{p}_{sb}")
                   for sb in range(NQB)] for p in range(PAIRS)]
            qt = [[pp.tile([128, QB], DT, tag=f"qt{p}_{sb}", name=f"qt{p}_{sb}")
                   for sb in range(NQB)] for p in range(PAIRS)]
            vt = [pp.tile([128, VW * HPC], DT, tag=f"vt{s}", name=f"vt{s}")
                  for s in range(SC)]
            # ones columns for the denominator trick: memset only col 0 of
            # each per-head block (cols 1-63 feed PSUM rows that are never
            # read); projections only ever write the 64 value columns.
            for s in range(SC):
                nc.vector.memset(
                    vt[s][:].rearrange("p (h w) -> p h w", w=VW)[:, :, 0:1],
                    1.0)

            def _mk_chain():
                def chain(name, width, lhs_of_e, rhs_of_e, copy_out):
                    st = {}
                    def mk(e):
                        def step():
                            if e == 0:
                                st["ps"] = psA.tile(
                                    [128, width], F32, tag="misc", bufs=2,
                                    name=name)
                            nc.tensor.matmul(st["ps"][:],
                                             lhs_of_e(e), rhs_of_e(e),
                                             start=(e == 0), stop=(e == EC - 1))
                        return step
                    for e in range(EC):
                        yield mk(e)
                    yield lambda: copy_out(st["ps"])
                return chain

            def kq_pair_ops(sb, p, chain=None):
                chain = chain or _mk_chain()
                if sb == 0:
                    # startup window: ACT is otherwise idle here
                    kcopy = lambda ps, p=p, sb=sb: nc.scalar.activation(
                        kt[p][sb][:], ps[:], Iden, bias=bundle[:, 3 + p:4 + p])
                    qcopy = lambda ps, p=p, sb=sb: nc.scalar.activation(
                        qt[p][sb][:], ps[:], Iden, bias=bundle[:, p:p + 1])
                else:
                    kcopy = lambda ps, p=p, sb=sb: nc.vector.tensor_scalar_add(
                        kt[p][sb][:], ps[:], bundle[:, 3 + p:4 + p])
                    qcopy = lambda ps, p=p, sb=sb: nc.vector.tensor_scalar_add(
                        qt[p][sb][:], ps[:], bundle[:, p:p + 1])
                yield from chain(
                    f"kps{p}_{sb}", QB,
                    lambda e, p=p: wk[p][:, e * 128:(e + 1) * 128],
                    lambda e, sb=sb: xchunk(e, sb), kcopy)
                yield from chain(
                    f"qps{p}_{sb}", QB,
                    lambda e, p=p: wq[p][:, e * 128:(e + 1) * 128],
                    lambda e, sb=sb: xchunk(e, sb), qcopy)

            def kq_ops(sb, chain=None):
                for p in range(PAIRS):
                    yield from kq_pair_ops(sb, p, chain)

            def v_chunk_ops(sb, s, chain=None):
                chain = chain or _mk_chain()
                def vcopy(ps, s=s, sb=sb):
                    dst = vt[s][:].rearrange(
                        "p (h w) -> p h w", w=VW)[:, :, VO:VO + VH]
                    srcv = ps[:].rearrange("p (h w) -> p h w", w=VH)
                    if sb == 0:
                        nc.scalar.activation(dst, srcv, Iden)
                    else:
                        nc.vector.tensor_copy(dst, srcv)
                yield from chain(
                    f"vps{s}", VH * HPC,
                    lambda e, sb=sb, s=s: xchunk(e, sb, (s % 4) * 128, 128),
                    lambda e: wv[e], vcopy)

            def v_ops(sb, chain=None):
                for s in range(4 * sb, 4 * sb + 4):
                    yield from v_chunk_ops(sb, s, chain)

            def make_normalize(qb, zpair):
                def release(head, zab, last=False):
                    # Emitted right at the pair end so the zab PSUM banks
                    # free before the next pair's first PV (tag z rotates 2
                    # buffers per pair): one f32 copy of the z rows plus the
                    # fast reciprocal of the PSUM den row (partition 0, as
                    # reciprocal_approx_fast requires). The broadcast runs on
                    # gpsimd immediately (nothing queued behind it); the
                    # normalize multiply is deferred so the DVE queue never
                    # head-of-line blocks on the broadcast.
                    if last:
                        zsb = zab[VO:VO + VH, :]
                    else:
                        zt = wp.tile([VH, QB], F32, tag="zc", bufs=6,
                                     name=f"zsb{qb}_{head}")
                        nc.vector.tensor_copy(zt[:], zab[VO:VO + VH, :])
                        zsb = zt[:]
                    recipf = wp.tile([1, QB], F32, tag="recipf", bufs=6,
                                     name=f"recipf{qb}_{head}")
                    nc.vector.reciprocal_approx_fast(recipf[:], zab[0:1, :])
                    bcast = wp.tile([64, QB], F32, tag="bcast", bufs=6,
                                    name=f"bcast{qb}_{head}")
                    nc.gpsimd.partition_broadcast(bcast[:], recipf[:])
                    return zsb, bcast

                def normalize(head, zsb, bcast, last=False):
                    p, sub = head // 2, head % 2
                    hsl = slice(sub * 64, sub * 64 + 64)
                    zt = zpair[p]
                    # last pair: per-head tiles (rows 0-63 each) so pass2's
                    # first contraction half starts before head 5 normalizes
                    dst = zt[sub][0:64, :] if isinstance(zt, list) else zt[hsl, :]
                    nc.vector.tensor_mul(dst, zsb, bcast[:])
                return release, normalize

            def attention(qb, drain=None, late=None, last_pair_drain=None,
                          zpair_override=None, pre_pv=None):
                nkb = 4 * qb + 4
                dq = list(drain) if drain is not None else []
                pre_pv = dict(pre_pv) if pre_pv else {}

                def run_pre_pv(kb):
                    for step in pre_pv.pop(kb, ()):
                        step()
                iters = [PAIRS * max(nkb - 1, 1), 0]

                def drain_some():
                    if not dq:
                        return
                    n = max(1, -(-len(dq) // max(iters[0] - iters[1], 1)))
                    for _ in range(n):
                        if dq:
                            dq.pop(0)()
                    iters[1] += 1
                zpair = zpair_override or [
                    zp.tile([128, QB], DT, tag=f"zp{p}", name=f"zp{p}_{qb}")
                    for p in range(PAIRS)]
                release, normalize = make_normalize(qb, zpair)
                pending = []
                for p in range(PAIRS):
                    zab = [psA.tile([VO + VH, QB], F32, tag="z", bufs=2,
                                    name=f"zps{qb}_{2 * p + s}") for s in range(2)]

                    def qk(kb):
                        # both heads of the pair, concurrent via PE row groups;
                        # on diagonal blocks only the causally-live columns.
                        o = kb - 4 * qb
                        lo = o * 128 if o > 0 else 0
                        sps = psA.tile([KB, 2 * QB], F32, tag="s", bufs=2,
                                       name=f"sps{qb}_{p}_{kb}")
                        ktt = kt[p][kb // 4]
                        ksl = slice((kb % 4) * KB, (kb % 4 + 1) * KB)
                        nc.tensor.matmul(
                            sps[:, lo:QB], ktt[0:64, ksl],
                            qt[p][qb][0:64, lo:QB],
                            start=True, stop=True, tile_position=(0, 0))
                        nc.tensor.matmul(
                            sps[:, QB + lo:2 * QB], ktt[64:128, ksl],
                            qt[p][qb][64:128, lo:QB],
                            start=True, stop=True, tile_position=(64, 0))
                        return sps, lo

                    def pv(kb, sps, lo):
                        o = kb - 4 * qb
                        pt = wp.tile([KB, 2 * QB], DT, tag="p", bufs=6,
                                     name=f"pt{qb}_{p}_{kb}")
                        if lo == 0:
                            nc.scalar.activation(pt[:], sps[:], Exp,
                                                 scale=float(SCALE))
                        else:
                            s3 = sps[:].rearrange(
                                "p (h s) -> p h s", h=2)[:, :, lo:]
                            p3 = pt[:].rearrange(
                                "p (h s) -> p h s", h=2)[:, :, lo:]
                            nc.scalar.activation(p3, s3, Exp,
                                                 scale=float(SCALE))
                        if o >= 0:  # diagonal: mask the 128-wide tri chunk
                            pm = pt[:].rearrange(
                                "p (h s) -> p h s", h=2)[:, :, o * 128:(o + 1) * 128]
                            nc.vector.tensor_mul(pm, pm, mask3)
                        for s in range(2):
                            nc.tensor.matmul(
                                zab[s][:, lo:QB],
                                vt[kb][:, (2 * p + s) * VW:(2 * p + s + 1) * VW],
                                pt[:, s * QB + lo:(s + 1) * QB],
                                start=(kb == 0), stop=(kb == nkb - 1))

                    prev = qk(0)
                    for kb in range(1, nkb):
                        cur = qk(kb)
                        run_pre_pv(kb - 1)
                        drain_some()
                        pv(kb - 1, *prev)
                        prev = cur
                        if kb == 2:
                            for args in pending:
                                normalize(*args)
                            pending = []
                            if p == PAIRS - 1 and last_pair_drain is not None:
                                dq.extend(last_pair_drain)
                    run_pre_pv(nkb - 1)
                    pv(nkb - 1, *prev)
                    drain_some()

                    last = (qb == NQB - 1 and p == PAIRS - 1)
                    for s in range(2):
                        head = 2 * p + s
                        zsb, bcast = release(head, zab[s], last)
                        pending.append((head, zsb, bcast, last))
                for args in pending:
                    normalize(*args)
                while dq:
                    dq.pop(0)()
                if late is not None:
                    for step in late:
                        step()
                return outproj_ops(qb, zpair)

            def outproj_split(qb, zpair):
                """qb=3 variant: p0+p1 partials run early (PE bubbles during
                the last pair), only the short p2 pass waits on the final
                normalize."""
                qsl = slice(qb * QB, (qb + 1) * QB)
                partial = [None] * EC

                def pass1():
                    for e in range(EC):
                        st = {}
                        def mk(e, p, st=st):
                            def step():
                                if p == 0:
                                    st["ps"] = psA.tile(
                                        [128, QB], F32, tag="misc", bufs=2,
                                        name=f"opsa{qb}_{e}")
                                nc.tensor.matmul(
                                    st["ps"][:], wo[p][:, e * 128:(e + 1) * 128],
                                    zpair[p][:], start=(p == 0), stop=(p == 1))
                            return step
                        yield mk(e, 0)
                        yield mk(e, 1)
                        def fin(e, st=st):
                            def step():
                                t = op.tile([128, QB], F32, tag=f"partial{e}",
                                            bufs=1, name=f"partial{qb}_{e}")
                                partial[e] = t
                                nc.vector.tensor_scalar_add(
                                    t[:], st["ps"][:], bundle[:, 6 + e:7 + e])
                            return step
                        yield fin(e)

                def pass2():
                    # split by head: the head-4 half of the contraction runs
                    # while head 5 is still normalizing
                    sts = {}
                    def mka(e):
                        def step():
                            sts[e] = psA.tile([128, QB], F32, tag="misc",
                                              bufs=2, name=f"opsb{qb}_{e}")
                            nc.tensor.matmul(
                                sts[e][:], wo[2][0:64, e * 128:(e + 1) * 128],
                                zpair[2][0][0:64, :], start=True, stop=False,
                                tile_position=(0, 0))
                        return step
                    def mkb(e):
                        def step():
                            nc.tensor.matmul(
                                sts[e][:], wo2hi[0:64, e * 128:(e + 1) * 128],
                                zpair[2][1][0:64, :], start=False, stop=True,
                                tile_position=(0, 0))
                        return step
                    def fin(e):
                        def step():
                            osb = op.tile([128, QB], DT, tag="osb",
                                          name=f"osb{qb}_{e}")
                            nc.vector.tensor_add(osb[:], sts[e][:],
                                                 partial[e][:])
                            nc.sync.dma_start(
                                d_out[e * 128:(e + 1) * 128, qsl], osb[:])
                        return step
                    yield mka(0)
                    yield mka(1)
                    for e in range(EC):
                        yield mkb(e)
                        yield fin(e)
                        if e + 2 < EC:
                            yield mka(e + 2)
                return pass1, pass2

            def outproj_ops(qb, zpair):
                qsl = slice(qb * QB, (qb + 1) * QB)
                for e in range(EC):
                    st = {}
                    def mk(e, p):
                        def step():
                            if p == 0:
                                st["ps"] = psA.tile([128, QB], F32, tag="misc",
                                                    bufs=2, name=f"ops{qb}_{e}")
                            nc.tensor.matmul(
                                st["ps"][:], wo[p][:, e * 128:(e + 1) * 128],
                                zpair[p][:],
                                start=(p == 0), stop=(p == PAIRS - 1))
                        return step
                    for p in range(PAIRS):
                        yield mk(e, p)
                    def fin(e):
                        def step():
                            osb = op.tile([128, QB], DT, tag="osb",
                                          name=f"osb{qb}_{e}")
                            nc.vector.tensor_scalar_add(
                                osb[:], st["ps"][:], bundle[:, 6 + e:7 + e])
                            nc.sync.dma_start(d_out[e * 128:(e + 1) * 128, qsl],
                                              osb[:])
                        return step
                    yield fin(e)

            # startup: K/Q of pair 0 only, then the attention loop starts;
            # the qb0 V chains drain at the FRONT of the w0 queue (one full
            # 7-closure V chain per drain slot beats each pv into the PE
            # FIFO, so no deadlock), followed by the rest of proj(0/1).
            for step in kq_pair_ops(0, 0):
                step()
            w0_drain = (list(kq_pair_ops(0, 1)) + list(kq_pair_ops(0, 2))
                        + list(kq_ops(1)))
            op0 = list(attention(0, drain=iter(w0_drain), late=v_ops(1),
                                 pre_pv={s: list(v_chunk_ops(0, s))
                                         for s in range(4)}))
            op1 = list(attention(1, drain=iter(kq_ops(2)), late=v_ops(2)))

            # partition-0 copy of wo[2] rows 64-127 so pass2's second half can
            # run in PE row group 0 (serialized with the first half — avoids
            # a concurrent-accumulate drain race into the same PSUM).
            # Emitted here (not at startup) so the DVE queue never head-of-line
            # blocks on the late wo DMA.
            wo2hi = pp.tile([128, E], DT, tag="wo2hi", name="wo2hi")
            nc.vector.tensor_copy(wo2hi[0:64, :], wo_all[64:128, 2 * E:3 * E])

            op2 = list(attention(2, drain=iter(kq_ops(3)), late=v_ops(3)))

            zpair_last = [zp.tile([128, QB], DT, tag=f"zpL{p}",
                                  name=f"zpL{p}") for p in range(2)]
            zpair_last.append(
                [zp.tile([128, QB], DT, tag="zp2h", bufs=2,
                         name=f"zpL2_{s}") for s in range(2)])
            pass1, pass2 = outproj_split(3, zpair_last)
            pass1_steps = list(pass1())
            attention(3, drain=iter(op0 + op1 + op2),
                      last_pair_drain=iter(pass1_steps[:9]),
                      zpair_override=zpair_last)
            for step in pass1_steps[9:]:
                step()
            for step in pass2():
                step()

    nc.compile()
    return nc


def _get_nc():
    if _g["nc"] is None:
        _g["nc"] = _build()
    return _g["nc"]


def _make_in_maps(inputs):
    x = np.asarray(inputs["normalized_resid_pre"], dtype=np.float32)
    W_Q = np.asarray(inputs["W_Q"], dtype=np.float32)
    W_K = np.asarray(inputs["W_K"], dtype=np.float32)
    W_V = np.asarray(inputs["W_V"], dtype=np.float32)
    W_O = np.asarray(inputs["W_O"], dtype=np.float32)
    b_Q = np.asarray(inputs["b_Q"], dtype=np.float32)
    b_K = np.asarray(inputs["b_K"], dtype=np.float32)
    b_V = np.asarray(inputs["b_V"], dtype=np.float32)
    dt = _np_dt()

    # 0/1 keep-mask for the 128-wide diagonal triangle chunk, duplicated for
    # the two heads of a pair: keep when k-within-chunk <= q-within-chunk.
    tri = np.tril(np.ones((KB, KB), dtype=np.float32)).T  # [dk, dq] keep dk<=dq
    mask = np.concatenate([tri, tri], axis=1).astype(dt)  # [128, 256]

    in_maps = []
    for c in range(8):
        b = c // 2
        hs = (c % 2) * HPC
        heads = list(range(hs, hs + HPC))
        def pack(w):
            # [E, C] -> [128, EC*C] with column block e holding rows e*128..
            C = w.shape[1]
            return np.ascontiguousarray(
                w.reshape(EC, 128, C).transpose(1, 0, 2).reshape(128, EC * C))

        wq = np.concatenate(
            [pack(np.concatenate([W_Q[heads[2 * p]], W_Q[heads[2 * p + 1]]], axis=1))
             for p in range(PAIRS)], axis=1)             # [128, 3*768]
        wk = np.concatenate(
            [pack(np.concatenate([W_K[heads[2 * p]], W_K[heads[2 * p + 1]]], axis=1))
             for p in range(PAIRS)], axis=1)
        wv = pack(np.concatenate([W_V[h] for h in heads], axis=1))  # [128, 6*384]
        # wo per pair packed [128, 768]: wo[p, e] = W_O_pair[p-th row, e]
        wo = np.concatenate(
            [np.concatenate([W_O[heads[2 * p]], W_O[heads[2 * p + 1]]], axis=0)
             .reshape(128, E)
             for p in range(PAIRS)], axis=1)             # [128, 3*768]

        # x prepack: tile (quarter, half) = [128, 3*512] with
        # cols c*512+s = xT[(half*3+c)*128+p, quarter*512+s]
        xb = x[b]                                        # [S, E]
        # [quarter, half, 3, 128, 512] -> [128, quarter*half*3*512]
        xr = xb.reshape(NQB, QB, 2, 3, 128).transpose(4, 0, 2, 3, 1)
        xp = np.ascontiguousarray(xr.reshape(128, NQB * 2 * 3 * QB))

        # bias bundle [128, 12]: cols 0-2 bQ pairs, 3-5 bK pairs, 6-11 bO_eff
        bundle = np.zeros((128, 12), dtype=np.float32)
        for p in range(PAIRS):
            bundle[:, p] = np.concatenate(
                [b_Q[heads[2 * p]], b_Q[heads[2 * p + 1]]])
            bundle[:, 3 + p] = np.concatenate(
                [b_K[heads[2 * p]], b_K[heads[2 * p + 1]]])
        # fold b_V into b_O: out += sum_h Wo[h] @ bV[h]  (sum(P)/den == 1)
        # (b_O itself is applied on the host after the gather)
        bo_eff = np.einsum(
            "nhe,nh->e", W_O[heads], b_V[heads]).astype(np.float32)
        bundle[:, 6:12] = bo_eff.reshape(EC, 128).T

        in_maps.append({
            "xp": xp.astype(dt),
            "wq": wq.astype(dt), "wk": wk.astype(dt),
            "wv": wv.astype(dt), "wo": wo.astype(dt),
            "bundle": bundle,
            "mask": mask,
        })
    return in_maps


def _gather(results, b_O):
    out = np.empty((B, S, E), dtype=np.float32)
    for b in range(B):
        acc = results[2 * b]["outT"].astype(np.float32) + \
              results[2 * b + 1]["outT"].astype(np.float32)
        out[b] = acc.T + b_O
    return out


def run(inputs, trace=False):
    """Returns (output, BassKernelResults)."""
    from concourse.bass_utils import run_bass_kernel_spmd

    if trace:
        _install_ntff_shim()
    nc = _get_nc()
    in_maps = _make_in_maps(inputs)
    res = run_bass_kernel_spmd(nc, in_maps, core_ids=list(range(8)), trace=trace)
    b_O = np.asarray(inputs["b_O"], dtype=np.float32)
    return _gather(res.results, b_O), res


def kernel(**inputs):
    out, _ = run(inputs, trace=False)
    return out


def _install_ntff_shim():
    """The agent image's antenv lacks axon_hooks; recreate it so
    run_bass_kernel_spmd(trace=True) can capture NTFF profiles."""
    import types, ctypes, contextlib

    if "antenv.axon_hooks" in sys.modules:
        return
    so_path = "/opt/axon/libaxon_pjrt.so"
    try:
        lib = ctypes.CDLL(so_path)
        lib.axon_start_nrt_profile.argtypes = [ctypes.POINTER(ctypes.c_int64),
                                              ctypes.c_size_t]
        lib.axon_start_nrt_profile.restype = ctypes.c_int64
        lib.axon_stop_nrt_profile.argtypes = [ctypes.c_char_p]
        lib.axon_stop_nrt_profile.restype = ctypes.c_int64
    except (OSError, AttributeError):
        return

    @contextlib.contextmanager
    def _hook(output_dir, device_ids):
        import jax

        jax.devices()
        if device_ids:
            ids = (ctypes.c_int64 * len(device_ids))(*device_ids)
            rc = lib.axon_start_nrt_profile(ids, len(device_ids))
        else:
            rc = lib.axon_start_nrt_profile(None, 0)
        if rc != 0:
            raise RuntimeError(f"axon_start_nrt_profile rc={rc}")
        try:
            yield
        finally:
            n = lib.axon_stop_nrt_profile(str(output_dir).encode())
            print(f"ntff profile: {n} file(s) -> {output_dir}", file=sys.stderr)

    mod = types.ModuleType("antenv.axon_hooks")
    mod.get_axon_ntff_profile_hook = lambda: _hook
    sys.modules["antenv.axon_hooks"] = mod
    # avoid S3 upload attempts from the trace post-processing
    from concourse import bass_utils as bu

    bu.upload_artifacts = lambda tmpdir: f"local:{tmpdir}"
